# revision 16
# baseline (speedup 1.0000x reference)
"""Trainium2 Bass kernel for a dense transformer block (attention + FFN).

Sharding: data-parallel over (batch, sequence-parity). 8 cores = 4 batches x 2
parity groups. Core c handles batch b = c//2 and the 128-row blocks of parity
p = c%2 (blocks p, p+2, ..., p+14) as query rows; K/V are computed for the
full sequence of the batch on-core (no collectives). The causal structure is
made SPMD-uniform by computing, for query block i, key blocks j <= 2i+1 and
masking with a small per-core multiplicative mask input.

On-chip layout: activations are kept feature-major (transposed) where matmuls
need them as stationary/moving operands; scores are computed transposed
(S^T[c,q]) so softmax probabilities feed the attention*V matmul directly with
no transpose, and the row-sum comes for free from an appended ones-column in
the V stationary operand.
"""
import sys

sys.path.insert(0, '/opt/trn_rl_repo')

import numpy as np
import ml_dtypes

import bass_rust
import concourse.bass as bass
import concourse.tile as tile
from concourse import mybir

P = 128
T = 2048
TQ = 1024
C = 768
H = 12
D = 64
FF = 3072
EO = C // P          # 6
MB = FF // P         # 24
NB = T // P          # 16
NQ = TQ // P         # 8
HP = H // 2          # 6

f32 = mybir.dt.float32
bf16 = mybir.dt.bfloat16
AF = mybir.ActivationFunctionType
ALU = mybir.AluOpType


def split_multiwait_instructions(nc):
    """The installed walrus build rejects any instruction carrying more than
    one sync wait; hoist extra waits onto NoOps inserted before it on the
    same (serial) engine."""
    n_fixed = 0
    for f in nc.m.functions:
        for bb in f.blocks:
            insts = bb.instructions
            new_insts = []
            dirty = False
            for inst in insts:
                si = inst.sync_info
                waits = list(si.on_wait) if si and si.on_wait else []
                if len(waits) > 1:
                    for j, w in enumerate(waits[:-1]):
                        nop = bass_rust.InstNoOp(
                            name=f"{inst.name}_sw{j}", ins=[], outs=[]
                        )
                        nop.engine = inst.engine
                        nop.sync_info = bass_rust.SyncInfo(
                            on_wait=[w], on_update=[]
                        )
                        new_insts.append(nop)
                    si.on_wait = waits[-1:]
                    dirty = True
                    n_fixed += 1
                new_insts.append(inst)
            if dirty:
                bb.instructions = new_insts
    return n_fixed


def build_program():
    """Build the single SPMD program (identical on all 8 cores)."""
    nc = bass.Bass("TRN2", target_bir_lowering=False, debug=False,
                   num_devices=8)

    xq_d = nc.declare_dram_parameter("xq", [TQ, C], f32, isOutput=False)
    xf_d = nc.declare_dram_parameter("xf", [T, C], f32, isOutput=False)
    wq_d = nc.declare_dram_parameter("wq", [C, C], bf16, isOutput=False)
    wk_d = nc.declare_dram_parameter("wk", [C, C], bf16, isOutput=False)
    wv_d = nc.declare_dram_parameter("wv", [C, C], bf16, isOutput=False)
    wp_d = nc.declare_dram_parameter("wp", [C, C], bf16, isOutput=False)
    w1_d = nc.declare_dram_parameter("w1", [C, FF], bf16, isOutput=False)
    w2_d = nc.declare_dram_parameter("w2", [FF, C], bf16, isOutput=False)
    bq_d = nc.declare_dram_parameter("bq", [P, HP], f32, isOutput=False)
    bk_d = nc.declare_dram_parameter("bk", [P, HP], f32, isOutput=False)
    bv_d = nc.declare_dram_parameter("bv", [1, C], bf16, isOutput=False)
    bp_d = nc.declare_dram_parameter("bp", [1, C], bf16, isOutput=False)
    b1_d = nc.declare_dram_parameter("b1", [P, MB], f32, isOutput=False)
    b2_d = nc.declare_dram_parameter("b2", [1, C], bf16, isOutput=False)
    mk_d = nc.declare_dram_parameter("mk", [P, 2, P], bf16, isOutput=False)
    s0_d = nc.declare_dram_parameter("s0", [P, 1], f32, isOutput=False)
    s1_d = nc.declare_dram_parameter("s1", [P, 1], f32, isOutput=False)
    e_d = nc.declare_dram_parameter("eab", [12, 24, P], bf16, isOutput=False)
    out_d = nc.declare_dram_parameter("out", [TQ, C], f32, isOutput=True)

    x2_d = nc.dram_tensor("x2scratch", [P, NQ, C], f32)

    with tile.TileContext(nc) as tc:
        # --- pool stack (released LIFO) -------------------------------
        pers = tc.alloc_tile_pool(name="pers", bufs=1)
        late = tc.alloc_tile_pool(name="late", bufs=1)   # outT, wp, h2T
        attnp = tc.alloc_tile_pool(name="attnp", bufs=1)  # KT, QT, V

        ones1 = pers.tile([1, P], bf16)
        nc.vector.memset(ones1[:], 1.0)
        ones64 = pers.tile([1, 64], bf16)
        nc.vector.memset(ones64[:], 1.0)
        bqc = pers.tile([P, HP], f32)
        nc.sync.dma_start(bqc[:], bq_d[:])
        bkc = pers.tile([P, HP], f32)
        nc.sync.dma_start(bkc[:], bk_d[:])
        bvr = pers.tile([1, C], bf16)
        nc.sync.dma_start(bvr[:], bv_d[:])
        bpr = pers.tile([1, C], bf16)
        nc.sync.dma_start(bpr[:], bp_d[:])
        b1c = pers.tile([P, MB], f32)
        nc.sync.dma_start(b1c[:], b1_d[:])
        b2r = pers.tile([1, C], bf16)
        nc.sync.dma_start(b2r[:], b2_d[:])
        masks = pers.tile([P, 2, P], bf16)
        nc.sync.dma_start(masks[:], mk_d[:])
        s0c = pers.tile([P, 1], f32)
        nc.sync.dma_start(s0c[:], s0_d[:])
        s1c = pers.tile([P, 1], f32)
        nc.sync.dma_start(s1c[:], s1_d[:])
        eabt = pers.tile([24, 12, P], bf16)
        nc.sync.dma_start(eabt[:], e_d.ap().rearrange("i k p -> k i p"))

        outT = late.tile([P, HP, TQ], bf16)
        wp_t = late.tile([P, EO, C], bf16)
        h2T_g = [late.tile([P, EO, 512], bf16, tag=f"h2T{g}", name=f"h2T{g}")
                 for g in range(2)]

        KT = attnp.tile([P, HP, T], bf16)
        QT = attnp.tile([P, HP, TQ], bf16)
        V = attnp.tile([P, NB, H, 65], bf16)
        nc.vector.memset(V[:], 1.0)

        def ln_block(x_ap, dst, col, nm, lnp, lns):
            # x_ap: [128, 768] (DRAM or SBUF) -> normalized bf16 rows,
            # transposed into dst[:, :, col:col+128]
            if x_ap.space == bass.MemorySpace.DRAM:
                x_t = lnp.tile([P, C], f32, tag="ln_x", name=f"lnx_{nm}")
                nc.sync.dma_start(x_t[:], x_ap)
            else:
                x_t = x_ap
            s1 = lns.tile([P, 1], f32, tag="ln_s1", name=f"s1_{nm}")
            nc.vector.tensor_reduce(s1[:], x_t[:], mybir.AxisListType.X,
                                    ALU.add)
            sq = lnp.tile([P, C], bf16, tag="ln_sq", name=f"sq_{nm}")
            s2 = lns.tile([P, 1], f32, tag="ln_s2", name=f"s2_{nm}")
            nc.scalar.activation(sq[:], x_t[:], AF.Square, accum_out=s2[:])
            mu = lns.tile([P, 1], f32, tag="ln_mu", name=f"mu_{nm}")
            nc.vector.tensor_scalar_mul(mu[:], s1[:], 1.0 / C)
            mu2 = lns.tile([P, 1], f32, tag="ln_mu2", name=f"mu2_{nm}")
            nc.vector.tensor_scalar(mu2[:], mu[:], mu[:], None, ALU.mult)
            ve = lns.tile([P, 1], f32, tag="ln_ve", name=f"ve_{nm}")
            nc.vector.tensor_scalar(ve[:], s2[:], 1.0 / C, 1e-5,
                                    ALU.mult, ALU.add)
            nc.vector.tensor_scalar(ve[:], ve[:], mu2[:], None, ALU.subtract)
            sd = lns.tile([P, 1], f32, tag="ln_sd", name=f"sd_{nm}")
            nc.scalar.activation(sd[:], ve[:], AF.Sqrt)
            rstd = lns.tile([P, 1], f32, tag="ln_rstd", name=f"rstd_{nm}")
            nc.vector.reciprocal(rstd[:], sd[:])
            nbias = lns.tile([P, 1], f32, tag="ln_nb", name=f"nb_{nm}")
            nc.vector.tensor_scalar(nbias[:], mu[:], rstd[:], -1.0,
                                    ALU.mult, ALU.mult)
            z = lnp.tile([P, C], bf16, tag="ln_z", name=f"z_{nm}")
            nc.scalar.activation(z[:], x_t[:], AF.Identity,
                                 bias=nbias[:], scale=rstd[:])
            eng = nc.scalar if (col // P) % 2 else nc.sync
            eng.dma_start_transpose(dst[:, :, col:col + P], z[:])

        # ---------------- Phase 1+2: LN1 and QKV projections -------------
        w13 = tc.alloc_tile_pool(name="w13", bufs=1)
        lnp = tc.alloc_tile_pool(name="lnp", bufs=3)
        lns = tc.alloc_tile_pool(name="lns", bufs=4)
        pq_ps = tc.alloc_tile_pool(name="pq_ps", bufs=3, space="PSUM")
        pv_ps = tc.alloc_tile_pool(name="pv_ps", bufs=3, space="PSUM")

        wq_t = w13.tile([P, EO, C], bf16)
        nc.sync.dma_start(wq_t[:], wq_d.ap().rearrange("(o p) f -> p o f", p=P))
        wk_t = w13.tile([P, EO, C], bf16)
        nc.sync.dma_start(wk_t[:], wk_d.ap().rearrange("(o p) f -> p o f", p=P))
        wv_t = w13.tile([P, EO, C], bf16)
        nc.sync.dma_start(wv_t[:], wv_d.ap().rearrange("(o p) f -> p o f", p=P))
        hT_g = [w13.tile([P, EO, 512], bf16, tag=f"hT{g}", name=f"hT{g}")
                for g in range(4)]
        hqT_g = [w13.tile([P, EO, 512], bf16, tag=f"hqT{g}", name=f"hqT{g}")
                 for g in range(2)]

        for b in range(NB):
            ln_block(xf_d.ap()[b * P:(b + 1) * P, :], hT_g[b // 4],
                     (b % 4) * P, f"f{b}", lnp, lns)
        # hqT = parity-selected columns of hT (s0/s1 are 1/0 per core parity)
        hq_tmp = w13.tile([P, EO, 512], bf16)
        for g in range(2):
            for half in range(2):
                blk = hT_g[2 * g + half][:].rearrange(
                    "p o (b two t) -> p o b two t", two=2, t=P)
                dstv = hqT_g[g][:, :, half * 256:(half + 1) * 256].rearrange(
                    "p o (b t) -> p o b t", t=P)
                tmpv = hq_tmp[:, :, half * 256:(half + 1) * 256].rearrange(
                    "p o (b t) -> p o b t", t=P)
                nc.vector.tensor_scalar(tmpv, blk[:, :, :, 0, :], s0c[:],
                                        None, ALU.mult)
                nc.vector.tensor_scalar(dstv, blk[:, :, :, 1, :], s1c[:],
                                        None, ALU.mult)
            nc.vector.tensor_tensor(hqT_g[g][:], hqT_g[g][:],
                                    hq_tmp[:], ALU.add)

        # Q^T (own rows) and K^T (full rows), 2 heads col-packed per tile
        for hp in range(HP):
            for qc in range(2):
                pq = pq_ps.tile([P, 512], f32, tag="pqkt", name=f"pq_{hp}_{qc}")
                for eo in range(EO):
                    for ab in range(2):
                        nc.tensor.matmul(
                            pq[64 * ab:64 * (ab + 1), :],
                            wq_t[:, eo, hp * P + 64 * ab:hp * P + 64 * (ab + 1)],
                            hqT_g[qc][:, eo, :],
                            start=(eo == 0), stop=(eo == EO - 1),
                            tile_position=(0, 64 * ab),
                            skip_group_check=True)
                nc.scalar.activation(QT[:, hp, qc * 512:(qc + 1) * 512],
                                     pq[:], AF.Identity,
                                     bias=bqc[:, hp:hp + 1])
            for cc in range(4):
                pk = pq_ps.tile([P, 512], f32, tag="pqkt", name=f"pk_{hp}_{cc}")
                for eo in range(EO):
                    for ab in range(2):
                        nc.tensor.matmul(
                            pk[64 * ab:64 * (ab + 1), :],
                            wk_t[:, eo, hp * P + 64 * ab:hp * P + 64 * (ab + 1)],
                            hT_g[cc][:, eo, :],
                            start=(eo == 0), stop=(eo == EO - 1),
                            tile_position=(0, 64 * ab),
                            skip_group_check=True)
                nc.scalar.activation(KT[:, hp, cc * 512:(cc + 1) * 512],
                                     pk[:], AF.Identity,
                                     bias=bkc[:, hp:hp + 1])

        # V natural [c, f] with a ones column at f=64 per head
        for cb in range(NB):
            for fo in range(2):
                pv = pv_ps.tile([P, 384], f32, tag="pv", name=f"pv_{cb}_{fo}")
                for eo in range(EO):
                    nc.tensor.matmul(
                        pv[:], hT_g[cb // 4][:, eo, (cb % 4) * P:(cb % 4 + 1) * P],
                        wv_t[:, eo, fo * 384:(fo + 1) * 384],
                        start=(eo == 0), stop=False)
                nc.tensor.matmul(pv[:], ones1[:],
                                 bvr[:, fo * 384:(fo + 1) * 384],
                                 start=False, stop=True)
                nc.scalar.activation(
                    V[:, cb, fo * 6:(fo + 1) * 6, 0:64],
                    pv[:].rearrange("p (g d) -> p g d", g=6),
                    AF.Identity)

        for _pool in (pv_ps, pq_ps, lns, lnp, w13):
            _pool.release()

        # prefetch the projection weight while attention runs
        nc.sync.dma_start(wp_t[:], wp_d.ap().rearrange("(o p) f -> p o f", p=P))

        # ---------------- Phase 3: attention -----------------------------
        pr = tc.alloc_tile_pool(name="pr", bufs=3)
        rsn = tc.alloc_tile_pool(name="rsn", bufs=2)
        psc_ps = tc.alloc_tile_pool(name="psc", bufs=2, space="PSUM")
        pso_ps = tc.alloc_tile_pool(name="pso", bufs=2, space="PSUM")

        # rs16_pad rows 0:24 hold bf16 row-sums; the rest are 1.0 filler so
        # the padded reciprocal stays finite.
        rs16_pad = rsn.tile([P, 512], bf16, name="rs16_pad")
        nc.vector.memset(rs16_pad[:], 1.0)

        for hp in range(HP):
            for qc in range(2):
                q0 = qc * 512
                poA = pso_ps.tile([P, 512], f32, tag="poA", name=f"poA_{hp}_{qc}")
                poB = pso_ps.tile([P, 512], f32, tag="poB", name=f"poB_{hp}_{qc}")
                po_t = [poA, poB]
                jmax = 8 if qc == 0 else 16
                for j in range(jmax):
                    qsj = (j // 2) * P
                    qs = max(qsj, q0)
                    off = qs - q0
                    N = 512 - off
                    psc = psc_ps.tile([P, 2, 512], f32, tag="psc",
                                      name=f"psc_{hp}_{qc}_{j}")
                    for ab in range(2):
                        nc.tensor.matmul(
                            psc[:, ab, off:off + N],
                            KT[64 * ab:64 * (ab + 1), hp, j * P:(j + 1) * P],
                            QT[64 * ab:64 * (ab + 1), hp, qs:qs + N],
                            start=True, stop=True,
                            tile_position=(64 * ab, 0))
                    probs = pr.tile([P, 2, 512], bf16, tag="probs",
                                    name=f"pb_{hp}_{qc}_{j}")
                    nc.scalar.activation(probs[:, :, off:off + N],
                                         psc[:, :, off:off + N],
                                         AF.Exp, scale=0.125)
                    if qs == qsj:
                        nc.vector.tensor_tensor(
                            probs[:, :, off:off + P],
                            probs[:, :, off:off + P],
                            masks[:, j % 2, None, :].to_broadcast(
                                (P, 2, P)), ALU.mult)
                    for ab in range(2):
                        nc.tensor.matmul(
                            po_t[ab][0:65, off:off + N],
                            V[:, j, 2 * hp + ab, :],
                            probs[:, ab, off:off + N],
                            start=(j == 0), stop=(j == jmax - 1))
                # evict unnormalized out^T and stash the row-sums (row 64)
                for ab in range(2):
                    nc.vector.tensor_copy(
                        out=outT[64 * ab:64 * (ab + 1), hp, q0:q0 + 512],
                        in_=po_t[ab][0:64, :])
                    rstmp = rsn.tile([1, 512], bf16, tag="rstmp",
                                     name=f"rst_{hp}_{qc}_{ab}")
                    nc.vector.tensor_copy(out=rstmp[:],
                                          in_=po_t[ab][64:65, :])
                    k = hp * 4 + qc * 2 + ab
                    nc.sync.dma_start(rs16_pad[k:k + 1, :], rstmp[:])

        for _pool in (pso_ps, psc_ps):
            _pool.release()

        # ---- deferred normalization of outT ------------------------------
        nrm_ps = tc.alloc_tile_pool(name="nrm", bufs=2, space="PSUM")
        rsT = rsn.tile([P, 4, P], bf16, name="rsT")
        for c in range(4):
            nc.scalar.dma_start_transpose(rsT[:, c, :],
                                          rs16_pad[:, c * P:(c + 1) * P])
        rsTf = rsn.tile([P, 4 * P], f32, name="rsTf")
        nc.vector.tensor_copy(out=rsTf[:], in_=rsT[:])
        recTf = rsn.tile([P, 4 * P], f32, name="recTf")
        nc.vector.reciprocal(recTf[:], rsTf[:])
        recT16 = rsn.tile([P, 4, P], bf16, name="recT16")
        nc.vector.tensor_copy(out=recT16[:], in_=recTf[:])
        rec16_pad = rsn.tile([P, 512], bf16, name="rec16_pad")
        for c in range(4):
            nc.scalar.dma_start_transpose(rec16_pad[:, c * P:(c + 1) * P],
                                          recT16[:, c, :])
        for hp in range(HP):
            for qc in range(2):
                pb = nrm_ps.tile([P, 512], f32, tag="pbn",
                                 name=f"pbn_{hp}_{qc}")
                nc.tensor.matmul(pb[:], eabt[:, hp * 2 + qc, :],
                                 rec16_pad[0:24, :],
                                 start=True, stop=True)
                nc.vector.tensor_tensor(
                    outT[:, hp, qc * 512:(qc + 1) * 512],
                    outT[:, hp, qc * 512:(qc + 1) * 512],
                    pb[:], ALU.mult)

        for _pool in (nrm_ps, rsn, pr):
            _pool.release()
        attnp.release()

        # -------- Phase 4: projection + residual + LN2 --------------------
        # FFN weights prefetch during proj (pool allocated below the proj
        # transients so it survives into the FFN phase)
        ffn = tc.alloc_tile_pool(name="ffn", bufs=1)
        w1_t = ffn.tile([P, EO, FF], bf16)
        for eo in range(EO):
            nc.scalar.dma_start(w1_t[:, eo, :], w1_d.ap()[eo * P:(eo + 1) * P, :])
        w2_t = ffn.tile([P, MB, C], bf16)
        for mp in range(MB):
            nc.scalar.dma_start(w2_t[:, mp, :], w2_d.ap()[mp * P:(mp + 1) * P, :])
        uT = ffn.tile([P, MB, TQ], bf16)

        lnp2 = tc.alloc_tile_pool(name="lnp2", bufs=3)
        lns2 = tc.alloc_tile_pool(name="lns2", bufs=4)
        xres = tc.alloc_tile_pool(name="xres", bufs=3)
        ppr_ps = tc.alloc_tile_pool(name="ppr", bufs=2, space="PSUM")

        for qb in range(NQ):
            xqb = xres.tile([P, C], f32, tag="xqb", name=f"xqb_{qb}")
            nc.sync.dma_start(xqb[:], xq_d.ap()[qb * P:(qb + 1) * P, :])
            x2b = xres.tile([P, C], f32, tag="x2b", name=f"x2b_{qb}")
            for fo in range(2):
                pp = ppr_ps.tile([P, 384], f32, tag="ppr", name=f"pp_{qb}_{fo}")
                for fp in range(EO):
                    nc.tensor.matmul(
                        pp[:], outT[:, fp, qb * P:(qb + 1) * P],
                        wp_t[:, fp, fo * 384:(fo + 1) * 384],
                        start=(fp == 0), stop=False)
                nc.tensor.matmul(pp[:], ones1[:],
                                 bpr[:, fo * 384:(fo + 1) * 384],
                                 start=False, stop=True)
                nc.vector.tensor_tensor(
                    x2b[:, fo * 384:(fo + 1) * 384], pp[:],
                    xqb[:, fo * 384:(fo + 1) * 384], ALU.add)
            nc.sync.dma_start(x2_d.ap()[:, qb, :], x2b[:])
            ln_block(x2b[:], h2T_g[qb // 4], (qb % 4) * P,
                     f"x2{qb}", lnp2, lns2)

        for _pool in (ppr_ps, xres, lns2, lnp2):
            _pool.release()

        # -------- Phase 5: FFN --------------------------------------------
        oup = tc.alloc_tile_pool(name="oup", bufs=3)
        pu_ps = tc.alloc_tile_pool(name="pu", bufs=2, space="PSUM")
        py_ps = tc.alloc_tile_pool(name="py", bufs=2, space="PSUM")

        for mb in range(MB):
            for qc2 in range(2):
                pu = pu_ps.tile([P, 512], f32, tag="pu", name=f"pu_{mb}_{qc2}")
                for eo in range(EO):
                    nc.tensor.matmul(
                        pu[:], w1_t[:, eo, mb * P:(mb + 1) * P],
                        h2T_g[qc2][:, eo, :],
                        start=(eo == 0), stop=(eo == EO - 1))
                nc.scalar.activation(
                    uT[:, mb, qc2 * 512:(qc2 + 1) * 512], pu[:],
                    AF.Relu, bias=b1c[:, mb:mb + 1])
        for qb in range(NQ):
            x2r = oup.tile([P, C], f32, tag="x2r", name=f"x2r_{qb}")
            nc.sync.dma_start(x2r[:], x2_d.ap()[:, qb, :])
            for fo in range(2):
                py = py_ps.tile([P, 384], f32, tag="py", name=f"py_{qb}_{fo}")
                for mp in range(MB):
                    nc.tensor.matmul(
                        py[:], uT[:, mp, qb * P:(qb + 1) * P],
                        w2_t[:, mp, fo * 384:(fo + 1) * 384],
                        start=(mp == 0), stop=False)
                nc.tensor.matmul(py[:], ones1[:],
                                 b2r[:, fo * 384:(fo + 1) * 384],
                                 start=False, stop=True)
                ot = oup.tile([P, 384], f32, tag="ot", name=f"ot_{qb}_{fo}")
                nc.vector.tensor_tensor(
                    ot[:], py[:], x2r[:, fo * 384:(fo + 1) * 384], ALU.add)
                nc.sync.dma_start(
                    out_d.ap()[qb * P:(qb + 1) * P,
                               fo * 384:(fo + 1) * 384], ot[:])

        for _pool in (py_ps, pu_ps, oup, ffn, late, pers):
            _pool.release()

    return nc


def prepare_in_maps(inputs):
    """Build the 8 per-core input maps from the full problem inputs."""
    x = np.asarray(inputs["x"], np.float32)
    wq = np.asarray(inputs["wq"], np.float32)
    wk = np.asarray(inputs["wk"], np.float32)
    wv = np.asarray(inputs["wv"], np.float32)
    w_proj = np.asarray(inputs["w_proj"], np.float32)
    b_proj = np.asarray(inputs["b_proj"], np.float32)
    w1 = np.asarray(inputs["w1"], np.float32)
    b1 = np.asarray(inputs["b1"], np.float32)
    w2 = np.asarray(inputs["w2"], np.float32)
    b2 = np.asarray(inputs["b2"], np.float32)
    g1 = np.asarray(inputs["ln1_g"], np.float32)
    be1 = np.asarray(inputs["ln1_b"], np.float32)
    g2 = np.asarray(inputs["ln2_g"], np.float32)
    be2 = np.asarray(inputs["ln2_b"], np.float32)

    bf = ml_dtypes.bfloat16
    wq_r = wq.transpose(1, 0, 2).reshape(C, C)       # [c, h*d]
    wk_r = wk.transpose(1, 0, 2).reshape(C, C)
    wv_r = wv.transpose(1, 0, 2).reshape(C, C)
    wq_g = (g1[:, None] * wq_r).astype(bf)
    wk_g = (g1[:, None] * wk_r).astype(bf)
    wv_g = (g1[:, None] * wv_r).astype(bf)
    bq = (be1 @ wq_r).reshape(HP, P).T.copy().astype(np.float32)   # [128, hp]
    bk = (be1 @ wk_r).reshape(HP, P).T.copy().astype(np.float32)
    bv = (be1 @ wv_r).reshape(1, C).astype(bf)
    w1_g = (g2[:, None] * w1).astype(bf)
    b1f = (b1 + be2 @ w1).reshape(MB, P).T.copy().astype(np.float32)  # [128, mb]
    wp16 = w_proj.astype(bf)
    w2_16 = w2.astype(bf)
    bp = b_proj.reshape(1, C).astype(bf)
    b2r = b2.reshape(1, C).astype(bf)

    ci = np.arange(P)[:, None]
    qi = np.arange(P)[None, :]
    tri = (ci <= qi).astype(np.float32)          # visible where c <= q
    m_par = [
        np.stack([tri, np.zeros((P, P), np.float32)], 0),   # parity 0
        np.stack([np.ones((P, P), np.float32), tri], 0),    # parity 1
    ]

    eab = np.zeros((12, 24, P), np.float32)
    for i in range(12):
        hp_, qc_ = i // 2, i % 2
        eab[i, hp_ * 4 + qc_ * 2 + 0, 0:64] = 1.0
        eab[i, hp_ * 4 + qc_ * 2 + 1, 64:128] = 1.0
    eab16 = eab.astype(bf)

    in_maps = []
    for core in range(8):
        b, p = core // 2, core % 2
        xf = np.ascontiguousarray(x[b])
        xq = np.ascontiguousarray(
            x[b].reshape(NB, P, C)[p::2].reshape(TQ, C))
        mk = np.ascontiguousarray(m_par[p].transpose(1, 0, 2)).astype(bf)
        s0 = np.full((P, 1), 1.0 - p, np.float32)
        s1 = np.full((P, 1), float(p), np.float32)
        in_maps.append({
            "xq": xq, "xf": xf,
            "wq": wq_g, "wk": wk_g, "wv": wv_g, "wp": wp16,
            "w1": w1_g, "w2": w2_16,
            "bq": bq, "bk": bk, "bv": bv, "bp": bp, "b1": b1f, "b2": b2r,
            "mk": mk, "s0": s0, "s1": s1, "eab": eab16,
        })
    return in_maps


def assemble_output(results):
    """Reassemble the 8 per-core [1024, 768] outputs into [4, 2048, 768]."""
    out = np.empty((4, T, C), np.float32)
    for core in range(8):
        b, p = core // 2, core % 2
        blocks = results[core]["out"].reshape(NQ, P, C)
        ov = out[b].reshape(NB, P, C)
        ov[p::2] = blocks
    return out


_CACHED_NC = None


def kernel(**inputs) -> np.ndarray:
    global _CACHED_NC
    from concourse.bass_utils import run_bass_kernel_spmd

    if _CACHED_NC is None:
        nc = build_program()
        split_multiwait_instructions(nc)
        _CACHED_NC = nc
    in_maps = prepare_in_maps(inputs)
    res = run_bass_kernel_spmd(_CACHED_NC, in_maps, list(range(8)))
    return assemble_output(res.results)


# revision 19
# speedup vs baseline: 1.0040x; 1.0040x over previous
"""Trainium2 Bass kernel for a dense transformer block (attention + FFN).

Sharding: data-parallel over (batch, sequence-parity). 8 cores = 4 batches x 2
parity groups. Core c handles batch b = c//2 and the 128-row blocks of parity
p = c%2 (blocks p, p+2, ..., p+14) as query rows; K/V are computed for the
full sequence of the batch on-core (no collectives). The causal structure is
made SPMD-uniform by computing, for query block i, key blocks j <= 2i+1 and
masking with a small per-core multiplicative mask input.

On-chip layout: activations are kept feature-major (transposed) where matmuls
need them as stationary/moving operands; scores are computed transposed
(S^T[c,q]) so softmax probabilities feed the attention*V matmul directly with
no transpose, and the row-sum comes for free from an appended ones-column in
the V stationary operand.
"""
import sys

sys.path.insert(0, '/opt/trn_rl_repo')

import numpy as np
import ml_dtypes

import bass_rust
import concourse.bass as bass
import concourse.tile as tile
from concourse import mybir

P = 128
T = 2048
TQ = 1024
C = 768
H = 12
D = 64
FF = 3072
EO = C // P          # 6
MB = FF // P         # 24
NB = T // P          # 16
NQ = TQ // P         # 8
HP = H // 2          # 6

f32 = mybir.dt.float32
bf16 = mybir.dt.bfloat16
AF = mybir.ActivationFunctionType
ALU = mybir.AluOpType


def split_multiwait_instructions(nc):
    """The installed walrus build rejects any instruction carrying more than
    one sync wait; hoist extra waits onto NoOps inserted before it on the
    same (serial) engine."""
    n_fixed = 0
    for f in nc.m.functions:
        for bb in f.blocks:
            insts = bb.instructions
            new_insts = []
            dirty = False
            for inst in insts:
                si = inst.sync_info
                waits = list(si.on_wait) if si and si.on_wait else []
                if len(waits) > 1:
                    for j, w in enumerate(waits[:-1]):
                        nop = bass_rust.InstNoOp(
                            name=f"{inst.name}_sw{j}", ins=[], outs=[]
                        )
                        nop.engine = inst.engine
                        nop.sync_info = bass_rust.SyncInfo(
                            on_wait=[w], on_update=[]
                        )
                        new_insts.append(nop)
                    si.on_wait = waits[-1:]
                    dirty = True
                    n_fixed += 1
                new_insts.append(inst)
            if dirty:
                bb.instructions = new_insts
    return n_fixed


def build_program():
    """Build the single SPMD program (identical on all 8 cores)."""
    nc = bass.Bass("TRN2", target_bir_lowering=False, debug=False,
                   num_devices=8)

    xq_d = nc.declare_dram_parameter("xq", [TQ, C], f32, isOutput=False)
    xf_d = nc.declare_dram_parameter("xf", [T, C], f32, isOutput=False)
    wq_d = nc.declare_dram_parameter("wq", [C, C], bf16, isOutput=False)
    wk_d = nc.declare_dram_parameter("wk", [C, C], bf16, isOutput=False)
    wv_d = nc.declare_dram_parameter("wv", [C, C], bf16, isOutput=False)
    wp_d = nc.declare_dram_parameter("wp", [C, C], bf16, isOutput=False)
    w1_d = nc.declare_dram_parameter("w1", [C, FF], bf16, isOutput=False)
    w2_d = nc.declare_dram_parameter("w2", [FF, C], bf16, isOutput=False)
    bq_d = nc.declare_dram_parameter("bq", [P, HP], f32, isOutput=False)
    bk_d = nc.declare_dram_parameter("bk", [P, HP], f32, isOutput=False)
    bv_d = nc.declare_dram_parameter("bv", [1, C], bf16, isOutput=False)
    bp_d = nc.declare_dram_parameter("bp", [1, C], bf16, isOutput=False)
    b1_d = nc.declare_dram_parameter("b1", [P, MB], f32, isOutput=False)
    b2_d = nc.declare_dram_parameter("b2", [1, C], bf16, isOutput=False)
    mk_d = nc.declare_dram_parameter("mk", [P, 2, P], bf16, isOutput=False)
    s0_d = nc.declare_dram_parameter("s0", [P, 1], f32, isOutput=False)
    s1_d = nc.declare_dram_parameter("s1", [P, 1], f32, isOutput=False)
    e_d = nc.declare_dram_parameter("eab", [12, 24, P], bf16, isOutput=False)
    out_d = nc.declare_dram_parameter("out", [TQ, C], f32, isOutput=True)

    x2_d = nc.dram_tensor("x2scratch", [P, NQ, C], f32)

    with tile.TileContext(nc) as tc:
        # --- pool stack (released LIFO) -------------------------------
        pers = tc.alloc_tile_pool(name="pers", bufs=1)
        late = tc.alloc_tile_pool(name="late", bufs=1)   # outT, wp, h2T
        attnp = tc.alloc_tile_pool(name="attnp", bufs=1)  # KT, QT, V

        ones1 = pers.tile([1, P], bf16)
        nc.vector.memset(ones1[:], 1.0)
        ones64 = pers.tile([1, 64], bf16)
        nc.vector.memset(ones64[:], 1.0)
        bqc = pers.tile([P, HP], f32)
        nc.sync.dma_start(bqc[:], bq_d[:])
        bkc = pers.tile([P, HP], f32)
        nc.sync.dma_start(bkc[:], bk_d[:])
        bvr = pers.tile([1, C], bf16)
        nc.sync.dma_start(bvr[:], bv_d[:])
        bpr = pers.tile([1, C], bf16)
        nc.sync.dma_start(bpr[:], bp_d[:])
        b1c = pers.tile([P, MB], f32)
        nc.sync.dma_start(b1c[:], b1_d[:])
        b2r = pers.tile([1, C], bf16)
        nc.sync.dma_start(b2r[:], b2_d[:])
        masks = pers.tile([P, 2, P], bf16)
        nc.sync.dma_start(masks[:], mk_d[:])
        s0c = pers.tile([P, 1], f32)
        nc.sync.dma_start(s0c[:], s0_d[:])
        s1c = pers.tile([P, 1], f32)
        nc.sync.dma_start(s1c[:], s1_d[:])
        eabt = pers.tile([24, 12, P], bf16)
        nc.sync.dma_start(eabt[:], e_d.ap().rearrange("i k p -> k i p"))

        outT = late.tile([P, HP, TQ], bf16)
        wp_t = late.tile([P, EO, C], bf16)
        h2T_g = [late.tile([P, EO, 512], bf16, tag=f"h2T{g}", name=f"h2T{g}")
                 for g in range(2)]

        KT = attnp.tile([P, HP, T], bf16)
        QT = attnp.tile([P, HP, TQ], bf16)
        V = attnp.tile([P, NB, H, 65], bf16)
        nc.vector.memset(V[:], 1.0)

        def ln_block(x_ap, dst, col, nm, lnp, lns, sq_on_dve=False,
                     warm=None):
            # x_ap: [128, 768] (DRAM or SBUF) -> normalized bf16 rows,
            # transposed into dst[:, :, col:col+128]
            if x_ap.space == bass.MemorySpace.DRAM:
                x_t = lnp.tile([P, C], f32, tag="ln_x", name=f"lnx_{nm}")
                nc.sync.dma_start(x_t[:], x_ap)
            else:
                x_t = x_ap
            s1 = lns.tile([P, 1], f32, tag="ln_s1", name=f"s1_{nm}")
            nc.vector.tensor_reduce(s1[:], x_t[:], mybir.AxisListType.X,
                                    ALU.add)
            sq = lnp.tile([P, C], bf16, tag="ln_sq", name=f"sq_{nm}")
            s2 = lns.tile([P, 1], f32, tag="ln_s2", name=f"s2_{nm}")
            nc.scalar.activation(sq[:], x_t[:], AF.Square, accum_out=s2[:])
            mu = lns.tile([P, 1], f32, tag="ln_mu", name=f"mu_{nm}")
            nc.vector.tensor_scalar_mul(mu[:], s1[:], 1.0 / C)
            mu2 = lns.tile([P, 1], f32, tag="ln_mu2", name=f"mu2_{nm}")
            nc.vector.tensor_scalar(mu2[:], mu[:], mu[:], None, ALU.mult)
            ve = lns.tile([P, 1], f32, tag="ln_ve", name=f"ve_{nm}")
            nc.vector.tensor_scalar(ve[:], s2[:], 1.0 / C, 1e-5,
                                    ALU.mult, ALU.add)
            nc.vector.tensor_scalar(ve[:], ve[:], mu2[:], None, ALU.subtract)
            sd = lns.tile([P, 1], f32, tag="ln_sd", name=f"sd_{nm}")
            nc.scalar.activation(sd[:], ve[:], AF.Sqrt)
            rstd = lns.tile([P, 1], f32, tag="ln_rstd", name=f"rstd_{nm}")
            nc.vector.reciprocal(rstd[:], sd[:])
            nbias = lns.tile([P, 1], f32, tag="ln_nb", name=f"nb_{nm}")
            nc.vector.tensor_scalar(nbias[:], mu[:], rstd[:], -1.0,
                                    ALU.mult, ALU.mult)
            z = lnp.tile([P, C], bf16, tag="ln_z", name=f"z_{nm}")
            nc.scalar.activation(z[:], x_t[:], AF.Identity,
                                 bias=nbias[:], scale=rstd[:])
            nc.sync.dma_start_transpose(dst[:, :, col:col + P], z[:])
            if warm is not None:
                # cheap dependency-chained matmul to keep the PE clock warm
                nc.tensor.matmul(warm[:, 0:512], ones1[:], z[0:1, 0:512],
                                 start=True, stop=True)

        # ---------------- Phase 1+2: LN1 and QKV projections -------------
        w13 = tc.alloc_tile_pool(name="w13", bufs=1)
        lnp = tc.alloc_tile_pool(name="lnp", bufs=3)
        lns = tc.alloc_tile_pool(name="lns", bufs=4)
        pq_ps = tc.alloc_tile_pool(name="pq_ps", bufs=3, space="PSUM")
        pv_ps = tc.alloc_tile_pool(name="pv_ps", bufs=3, space="PSUM")
        warm_ps = tc.alloc_tile_pool(name="warm_ps", bufs=1, space="PSUM")

        wq_t = w13.tile([P, EO, C], bf16)
        nc.sync.dma_start(wq_t[:], wq_d.ap().rearrange("(o p) f -> p o f", p=P))
        wk_t = w13.tile([P, EO, C], bf16)
        nc.sync.dma_start(wk_t[:], wk_d.ap().rearrange("(o p) f -> p o f", p=P))
        wv_t = w13.tile([P, EO, C], bf16)
        nc.sync.dma_start(wv_t[:], wv_d.ap().rearrange("(o p) f -> p o f", p=P))
        hT_g = [w13.tile([P, EO, 512], bf16, tag=f"hT{g}", name=f"hT{g}")
                for g in range(4)]
        hqT_g = [w13.tile([P, EO, 512], bf16, tag=f"hqT{g}", name=f"hqT{g}")
                 for g in range(2)]

        warm1 = warm_ps.tile([P, 512], f32, tag="warm", name="warm1")
        for b in range(NB):
            ln_block(xf_d.ap()[b * P:(b + 1) * P, :], hT_g[b // 4],
                     (b % 4) * P, f"f{b}", lnp, lns, warm=warm1)
        # hqT = parity-selected columns of hT (s0/s1 are 1/0 per core parity)
        hq_tmp = w13.tile([P, EO, 512], bf16)
        for g in range(2):
            for half in range(2):
                blk = hT_g[2 * g + half][:].rearrange(
                    "p o (b two t) -> p o b two t", two=2, t=P)
                dstv = hqT_g[g][:, :, half * 256:(half + 1) * 256].rearrange(
                    "p o (b t) -> p o b t", t=P)
                tmpv = hq_tmp[:, :, half * 256:(half + 1) * 256].rearrange(
                    "p o (b t) -> p o b t", t=P)
                nc.vector.tensor_scalar(tmpv, blk[:, :, :, 0, :], s0c[:],
                                        None, ALU.mult)
                nc.vector.tensor_scalar(dstv, blk[:, :, :, 1, :], s1c[:],
                                        None, ALU.mult)
            nc.vector.tensor_tensor(hqT_g[g][:], hqT_g[g][:],
                                    hq_tmp[:], ALU.add)

        # Q^T (own rows) and K^T (full rows), 2 heads col-packed per tile
        for hp in range(HP):
            for qc in range(2):
                pq = pq_ps.tile([P, 512], f32, tag="pqkt", name=f"pq_{hp}_{qc}")
                for eo in range(EO):
                    for ab in range(2):
                        nc.tensor.matmul(
                            pq[64 * ab:64 * (ab + 1), :],
                            wq_t[:, eo, hp * P + 64 * ab:hp * P + 64 * (ab + 1)],
                            hqT_g[qc][:, eo, :],
                            start=(eo == 0), stop=(eo == EO - 1),
                            tile_position=(0, 64 * ab),
                            skip_group_check=True)
                nc.scalar.activation(QT[:, hp, qc * 512:(qc + 1) * 512],
                                     pq[:], AF.Identity,
                                     bias=bqc[:, hp:hp + 1])
            for cc in range(4):
                pk = pq_ps.tile([P, 512], f32, tag="pqkt", name=f"pk_{hp}_{cc}")
                for eo in range(EO):
                    for ab in range(2):
                        nc.tensor.matmul(
                            pk[64 * ab:64 * (ab + 1), :],
                            wk_t[:, eo, hp * P + 64 * ab:hp * P + 64 * (ab + 1)],
                            hT_g[cc][:, eo, :],
                            start=(eo == 0), stop=(eo == EO - 1),
                            tile_position=(0, 64 * ab),
                            skip_group_check=True)
                nc.scalar.activation(KT[:, hp, cc * 512:(cc + 1) * 512],
                                     pk[:], AF.Identity,
                                     bias=bkc[:, hp:hp + 1])

        # V natural [c, f] with a ones column at f=64 per head
        for cb in range(NB):
            for fo in range(2):
                pv = pv_ps.tile([P, 384], f32, tag="pv", name=f"pv_{cb}_{fo}")
                for eo in range(EO):
                    nc.tensor.matmul(
                        pv[:], hT_g[cb // 4][:, eo, (cb % 4) * P:(cb % 4 + 1) * P],
                        wv_t[:, eo, fo * 384:(fo + 1) * 384],
                        start=(eo == 0), stop=False)
                nc.tensor.matmul(pv[:], ones1[:],
                                 bvr[:, fo * 384:(fo + 1) * 384],
                                 start=False, stop=True)
                nc.scalar.activation(
                    V[:, cb, fo * 6:(fo + 1) * 6, 0:64],
                    pv[:].rearrange("p (g d) -> p g d", g=6),
                    AF.Identity)

        for _pool in (warm_ps, pv_ps, pq_ps, lns, lnp, w13):
            _pool.release()

        # prefetch the projection weight while attention runs
        nc.sync.dma_start(wp_t[:], wp_d.ap().rearrange("(o p) f -> p o f", p=P))

        # ---------------- Phase 3: attention -----------------------------
        pr = tc.alloc_tile_pool(name="pr", bufs=3)
        rsn = tc.alloc_tile_pool(name="rsn", bufs=2)
        psc_ps = tc.alloc_tile_pool(name="psc", bufs=2, space="PSUM")
        pso_ps = tc.alloc_tile_pool(name="pso", bufs=2, space="PSUM")

        # rs16_pad rows 0:24 hold bf16 row-sums; the rest are 1.0 filler so
        # the padded reciprocal stays finite.
        rs16_pad = rsn.tile([P, 512], bf16, name="rs16_pad")
        nc.vector.memset(rs16_pad[:], 1.0)

        for hp in range(HP):
            for qc in range(2):
                q0 = qc * 512
                poA = pso_ps.tile([P, 512], f32, tag="poA", name=f"poA_{hp}_{qc}")
                poB = pso_ps.tile([P, 512], f32, tag="poB", name=f"poB_{hp}_{qc}")
                po_t = [poA, poB]
                jmax = 8 if qc == 0 else 16
                for j in range(jmax):
                    qsj = (j // 2) * P
                    qs = max(qsj, q0)
                    off = qs - q0
                    N = 512 - off
                    psc = psc_ps.tile([P, 2, 512], f32, tag="psc",
                                      name=f"psc_{hp}_{qc}_{j}")
                    for ab in range(2):
                        nc.tensor.matmul(
                            psc[:, ab, off:off + N],
                            KT[64 * ab:64 * (ab + 1), hp, j * P:(j + 1) * P],
                            QT[64 * ab:64 * (ab + 1), hp, qs:qs + N],
                            start=True, stop=True,
                            tile_position=(64 * ab, 0))
                    probs = pr.tile([P, 2, 512], bf16, tag="probs",
                                    name=f"pb_{hp}_{qc}_{j}")
                    nc.scalar.activation(probs[:, :, off:off + N],
                                         psc[:, :, off:off + N],
                                         AF.Exp, scale=0.125)
                    if qs == qsj:
                        nc.vector.tensor_tensor(
                            probs[:, :, off:off + P],
                            probs[:, :, off:off + P],
                            masks[:, j % 2, None, :].to_broadcast(
                                (P, 2, P)), ALU.mult)
                    for ab in range(2):
                        nc.tensor.matmul(
                            po_t[ab][0:65, off:off + N],
                            V[:, j, 2 * hp + ab, :],
                            probs[:, ab, off:off + N],
                            start=(j == 0), stop=(j == jmax - 1))
                # evict unnormalized out^T and stash the row-sums (row 64)
                for ab in range(2):
                    nc.vector.tensor_copy(
                        out=outT[64 * ab:64 * (ab + 1), hp, q0:q0 + 512],
                        in_=po_t[ab][0:64, :])
                    rstmp = rsn.tile([1, 512], bf16, tag="rstmp",
                                     name=f"rst_{hp}_{qc}_{ab}")
                    nc.vector.tensor_copy(out=rstmp[:],
                                          in_=po_t[ab][64:65, :])
                    k = hp * 4 + qc * 2 + ab
                    nc.sync.dma_start(rs16_pad[k:k + 1, :], rstmp[:])

        for _pool in (pso_ps, psc_ps):
            _pool.release()

        # ---- deferred normalization of outT ------------------------------
        nrm_ps = tc.alloc_tile_pool(name="nrm", bufs=2, space="PSUM")
        rsT = rsn.tile([P, 4, P], bf16, name="rsT")
        for c in range(4):
            nc.scalar.dma_start_transpose(rsT[:, c, :],
                                          rs16_pad[:, c * P:(c + 1) * P])
        rsTf = rsn.tile([P, 4 * P], f32, name="rsTf")
        nc.vector.tensor_copy(out=rsTf[:], in_=rsT[:])
        recTf = rsn.tile([P, 4 * P], f32, name="recTf")
        nc.vector.reciprocal(recTf[:], rsTf[:])
        recT16 = rsn.tile([P, 4, P], bf16, name="recT16")
        nc.vector.tensor_copy(out=recT16[:], in_=recTf[:])
        rec16_pad = rsn.tile([P, 512], bf16, name="rec16_pad")
        for c in range(4):
            nc.scalar.dma_start_transpose(rec16_pad[:, c * P:(c + 1) * P],
                                          recT16[:, c, :])
        for hp in range(HP):
            for qc in range(2):
                pb = nrm_ps.tile([P, 512], f32, tag="pbn",
                                 name=f"pbn_{hp}_{qc}")
                nc.tensor.matmul(pb[:], eabt[:, hp * 2 + qc, :],
                                 rec16_pad[0:24, :],
                                 start=True, stop=True)
                nc.vector.tensor_tensor(
                    outT[:, hp, qc * 512:(qc + 1) * 512],
                    outT[:, hp, qc * 512:(qc + 1) * 512],
                    pb[:], ALU.mult)

        for _pool in (nrm_ps, rsn, pr):
            _pool.release()
        attnp.release()

        # -------- Phase 4: projection + residual + LN2 --------------------
        # FFN weights prefetch during proj (pool allocated below the proj
        # transients so it survives into the FFN phase)
        ffn = tc.alloc_tile_pool(name="ffn", bufs=1)
        w1_t = ffn.tile([P, EO, FF], bf16)
        for eo in range(EO):
            nc.scalar.dma_start(w1_t[:, eo, :], w1_d.ap()[eo * P:(eo + 1) * P, :])
        w2_t = ffn.tile([P, MB, C], bf16)
        for mp in range(MB):
            nc.scalar.dma_start(w2_t[:, mp, :], w2_d.ap()[mp * P:(mp + 1) * P, :])
        uT = ffn.tile([P, MB, TQ], bf16)

        lnp2 = tc.alloc_tile_pool(name="lnp2", bufs=3)
        lns2 = tc.alloc_tile_pool(name="lns2", bufs=4)
        xres = tc.alloc_tile_pool(name="xres", bufs=3)
        ppr_ps = tc.alloc_tile_pool(name="ppr", bufs=2, space="PSUM")
        warm2_ps = tc.alloc_tile_pool(name="warm2_ps", bufs=1, space="PSUM")

        warm2 = warm2_ps.tile([P, 512], f32, tag="warm2", name="warm2")
        for qb in range(NQ):
            xqb = xres.tile([P, C], f32, tag="xqb", name=f"xqb_{qb}")
            nc.sync.dma_start(xqb[:], xq_d.ap()[qb * P:(qb + 1) * P, :])
            x2b = xres.tile([P, C], f32, tag="x2b", name=f"x2b_{qb}")
            for fo in range(2):
                pp = ppr_ps.tile([P, 384], f32, tag="ppr", name=f"pp_{qb}_{fo}")
                for fp in range(EO):
                    nc.tensor.matmul(
                        pp[:], outT[:, fp, qb * P:(qb + 1) * P],
                        wp_t[:, fp, fo * 384:(fo + 1) * 384],
                        start=(fp == 0), stop=False)
                nc.tensor.matmul(pp[:], ones1[:],
                                 bpr[:, fo * 384:(fo + 1) * 384],
                                 start=False, stop=True)
                nc.vector.tensor_tensor(
                    x2b[:, fo * 384:(fo + 1) * 384], pp[:],
                    xqb[:, fo * 384:(fo + 1) * 384], ALU.add)
            nc.sync.dma_start(x2_d.ap()[:, qb, :], x2b[:])
            ln_block(x2b[:], h2T_g[qb // 4], (qb % 4) * P,
                     f"x2{qb}", lnp2, lns2, sq_on_dve=True,
                     warm=warm2)

        for _pool in (warm2_ps, ppr_ps, xres, lns2, lnp2):
            _pool.release()

        # -------- Phase 5: FFN --------------------------------------------
        oup = tc.alloc_tile_pool(name="oup", bufs=3)
        pu_ps = tc.alloc_tile_pool(name="pu", bufs=2, space="PSUM")
        py_ps = tc.alloc_tile_pool(name="py", bufs=2, space="PSUM")

        for mb in range(MB):
            for qc2 in range(2):
                pu = pu_ps.tile([P, 512], f32, tag="pu", name=f"pu_{mb}_{qc2}")
                for eo in range(EO):
                    nc.tensor.matmul(
                        pu[:], w1_t[:, eo, mb * P:(mb + 1) * P],
                        h2T_g[qc2][:, eo, :],
                        start=(eo == 0), stop=(eo == EO - 1))
                nc.vector.tensor_scalar(
                    uT[:, mb, qc2 * 512:(qc2 + 1) * 512], pu[:],
                    b1c[:, mb:mb + 1], 0.0, ALU.add, ALU.max)
        for qb in range(NQ):
            x2r = oup.tile([P, C], f32, tag="x2r", name=f"x2r_{qb}")
            nc.sync.dma_start(x2r[:], x2_d.ap()[:, qb, :])
            for fo in range(2):
                py = py_ps.tile([P, 384], f32, tag="py", name=f"py_{qb}_{fo}")
                for mp in range(MB):
                    nc.tensor.matmul(
                        py[:], uT[:, mp, qb * P:(qb + 1) * P],
                        w2_t[:, mp, fo * 384:(fo + 1) * 384],
                        start=(mp == 0), stop=False)
                nc.tensor.matmul(py[:], ones1[:],
                                 b2r[:, fo * 384:(fo + 1) * 384],
                                 start=False, stop=True)
                ot = oup.tile([P, 384], f32, tag="ot", name=f"ot_{qb}_{fo}")
                nc.vector.tensor_tensor(
                    ot[:], py[:], x2r[:, fo * 384:(fo + 1) * 384], ALU.add)
                nc.sync.dma_start(
                    out_d.ap()[qb * P:(qb + 1) * P,
                               fo * 384:(fo + 1) * 384], ot[:])

        for _pool in (py_ps, pu_ps, oup, ffn, late, pers):
            _pool.release()

    return nc


def prepare_in_maps(inputs):
    """Build the 8 per-core input maps from the full problem inputs."""
    x = np.asarray(inputs["x"], np.float32)
    wq = np.asarray(inputs["wq"], np.float32)
    wk = np.asarray(inputs["wk"], np.float32)
    wv = np.asarray(inputs["wv"], np.float32)
    w_proj = np.asarray(inputs["w_proj"], np.float32)
    b_proj = np.asarray(inputs["b_proj"], np.float32)
    w1 = np.asarray(inputs["w1"], np.float32)
    b1 = np.asarray(inputs["b1"], np.float32)
    w2 = np.asarray(inputs["w2"], np.float32)
    b2 = np.asarray(inputs["b2"], np.float32)
    g1 = np.asarray(inputs["ln1_g"], np.float32)
    be1 = np.asarray(inputs["ln1_b"], np.float32)
    g2 = np.asarray(inputs["ln2_g"], np.float32)
    be2 = np.asarray(inputs["ln2_b"], np.float32)

    bf = ml_dtypes.bfloat16
    wq_r = wq.transpose(1, 0, 2).reshape(C, C)       # [c, h*d]
    wk_r = wk.transpose(1, 0, 2).reshape(C, C)
    wv_r = wv.transpose(1, 0, 2).reshape(C, C)
    wq_g = (g1[:, None] * wq_r).astype(bf)
    wk_g = (g1[:, None] * wk_r).astype(bf)
    wv_g = (g1[:, None] * wv_r).astype(bf)
    bq = (be1 @ wq_r).reshape(HP, P).T.copy().astype(np.float32)   # [128, hp]
    bk = (be1 @ wk_r).reshape(HP, P).T.copy().astype(np.float32)
    bv = (be1 @ wv_r).reshape(1, C).astype(bf)
    w1_g = (g2[:, None] * w1).astype(bf)
    b1f = (b1 + be2 @ w1).reshape(MB, P).T.copy().astype(np.float32)  # [128, mb]
    wp16 = w_proj.astype(bf)
    w2_16 = w2.astype(bf)
    bp = b_proj.reshape(1, C).astype(bf)
    b2r = b2.reshape(1, C).astype(bf)

    ci = np.arange(P)[:, None]
    qi = np.arange(P)[None, :]
    tri = (ci <= qi).astype(np.float32)          # visible where c <= q
    m_par = [
        np.stack([tri, np.zeros((P, P), np.float32)], 0),   # parity 0
        np.stack([np.ones((P, P), np.float32), tri], 0),    # parity 1
    ]

    eab = np.zeros((12, 24, P), np.float32)
    for i in range(12):
        hp_, qc_ = i // 2, i % 2
        eab[i, hp_ * 4 + qc_ * 2 + 0, 0:64] = 1.0
        eab[i, hp_ * 4 + qc_ * 2 + 1, 64:128] = 1.0
    eab16 = eab.astype(bf)

    in_maps = []
    for core in range(8):
        b, p = core // 2, core % 2
        xf = np.ascontiguousarray(x[b])
        xq = np.ascontiguousarray(
            x[b].reshape(NB, P, C)[p::2].reshape(TQ, C))
        mk = np.ascontiguousarray(m_par[p].transpose(1, 0, 2)).astype(bf)
        s0 = np.full((P, 1), 1.0 - p, np.float32)
        s1 = np.full((P, 1), float(p), np.float32)
        in_maps.append({
            "xq": xq, "xf": xf,
            "wq": wq_g, "wk": wk_g, "wv": wv_g, "wp": wp16,
            "w1": w1_g, "w2": w2_16,
            "bq": bq, "bk": bk, "bv": bv, "bp": bp, "b1": b1f, "b2": b2r,
            "mk": mk, "s0": s0, "s1": s1, "eab": eab16,
        })
    return in_maps


def assemble_output(results):
    """Reassemble the 8 per-core [1024, 768] outputs into [4, 2048, 768]."""
    out = np.empty((4, T, C), np.float32)
    for core in range(8):
        b, p = core // 2, core % 2
        blocks = results[core]["out"].reshape(NQ, P, C)
        ov = out[b].reshape(NB, P, C)
        ov[p::2] = blocks
    return out


_CACHED_NC = None


def kernel(**inputs) -> np.ndarray:
    global _CACHED_NC
    from concourse.bass_utils import run_bass_kernel_spmd

    if _CACHED_NC is None:
        nc = build_program()
        split_multiwait_instructions(nc)
        _CACHED_NC = nc
    in_maps = prepare_in_maps(inputs)
    res = run_bass_kernel_spmd(_CACHED_NC, in_maps, list(range(8)))
    return assemble_output(res.results)


# revision 20
# speedup vs baseline: 1.0171x; 1.0130x over previous
"""Trainium2 Bass kernel for a dense transformer block (attention + FFN).

Sharding: data-parallel over (batch, sequence-parity). 8 cores = 4 batches x 2
parity groups. Core c handles batch b = c//2 and the 128-row blocks of parity
p = c%2 (blocks p, p+2, ..., p+14) as query rows; K/V are computed for the
full sequence of the batch on-core (no collectives). The causal structure is
made SPMD-uniform by computing, for query block i, key blocks j <= 2i+1 and
masking with a small per-core multiplicative mask input.

On-chip layout: activations are kept feature-major (transposed) where matmuls
need them as stationary/moving operands; scores are computed transposed
(S^T[c,q]) so softmax probabilities feed the attention*V matmul directly with
no transpose, and the row-sum comes for free from an appended ones-column in
the V stationary operand.
"""
import sys

sys.path.insert(0, '/opt/trn_rl_repo')

import numpy as np
import ml_dtypes

import bass_rust
import concourse.bass as bass
import concourse.tile as tile
from concourse import mybir

P = 128
T = 2048
TQ = 1024
C = 768
H = 12
D = 64
FF = 3072
EO = C // P          # 6
MB = FF // P         # 24
NB = T // P          # 16
NQ = TQ // P         # 8
HP = H // 2          # 6

f32 = mybir.dt.float32
bf16 = mybir.dt.bfloat16
AF = mybir.ActivationFunctionType
ALU = mybir.AluOpType


def split_multiwait_instructions(nc):
    """The installed walrus build rejects any instruction carrying more than
    one sync wait; hoist extra waits onto NoOps inserted before it on the
    same (serial) engine."""
    n_fixed = 0
    for f in nc.m.functions:
        for bb in f.blocks:
            insts = bb.instructions
            new_insts = []
            dirty = False
            for inst in insts:
                si = inst.sync_info
                waits = list(si.on_wait) if si and si.on_wait else []
                if len(waits) > 1:
                    for j, w in enumerate(waits[:-1]):
                        nop = bass_rust.InstNoOp(
                            name=f"{inst.name}_sw{j}", ins=[], outs=[]
                        )
                        nop.engine = inst.engine
                        nop.sync_info = bass_rust.SyncInfo(
                            on_wait=[w], on_update=[]
                        )
                        new_insts.append(nop)
                    si.on_wait = waits[-1:]
                    dirty = True
                    n_fixed += 1
                new_insts.append(inst)
            if dirty:
                bb.instructions = new_insts
    return n_fixed


def build_program():
    """Build the single SPMD program (identical on all 8 cores)."""
    nc = bass.Bass("TRN2", target_bir_lowering=False, debug=False,
                   num_devices=8)

    xq_d = nc.declare_dram_parameter("xq", [TQ, C], f32, isOutput=False)
    xf_d = nc.declare_dram_parameter("xf", [T, C], f32, isOutput=False)
    wq_d = nc.declare_dram_parameter("wq", [C, C], bf16, isOutput=False)
    wk_d = nc.declare_dram_parameter("wk", [C, C], bf16, isOutput=False)
    wv_d = nc.declare_dram_parameter("wv", [C, C], bf16, isOutput=False)
    wp_d = nc.declare_dram_parameter("wp", [C, C], bf16, isOutput=False)
    w1_d = nc.declare_dram_parameter("w1", [C, FF], bf16, isOutput=False)
    w2_d = nc.declare_dram_parameter("w2", [FF, C], bf16, isOutput=False)
    bq_d = nc.declare_dram_parameter("bq", [P, HP], f32, isOutput=False)
    bk_d = nc.declare_dram_parameter("bk", [P, HP], f32, isOutput=False)
    bv_d = nc.declare_dram_parameter("bv", [1, C], bf16, isOutput=False)
    bp_d = nc.declare_dram_parameter("bp", [1, C], bf16, isOutput=False)
    b1_d = nc.declare_dram_parameter("b1", [P, MB], f32, isOutput=False)
    b2_d = nc.declare_dram_parameter("b2", [1, C], bf16, isOutput=False)
    mk_d = nc.declare_dram_parameter("mk", [P, 2, P], bf16, isOutput=False)
    s0_d = nc.declare_dram_parameter("s0", [P, 1], f32, isOutput=False)
    s1_d = nc.declare_dram_parameter("s1", [P, 1], f32, isOutput=False)
    e_d = nc.declare_dram_parameter("eab", [12, 24, P], bf16, isOutput=False)
    out_d = nc.declare_dram_parameter("out", [TQ, C], f32, isOutput=True)

    x2_d = nc.dram_tensor("x2scratch", [P, NQ, C], f32)

    with tile.TileContext(nc) as tc:
        # --- pool stack (released LIFO) -------------------------------
        pers = tc.alloc_tile_pool(name="pers", bufs=1)
        late = tc.alloc_tile_pool(name="late", bufs=1)   # outT, wp, h2T
        attnp = tc.alloc_tile_pool(name="attnp", bufs=1)  # KT, QT, V

        ones1 = pers.tile([1, P], bf16)
        nc.vector.memset(ones1[:], 1.0)
        ones64 = pers.tile([1, 64], bf16)
        nc.vector.memset(ones64[:], 1.0)
        bqc = pers.tile([P, HP], f32)
        nc.sync.dma_start(bqc[:], bq_d[:])
        bkc = pers.tile([P, HP], f32)
        nc.sync.dma_start(bkc[:], bk_d[:])
        bvr = pers.tile([1, C], bf16)
        nc.sync.dma_start(bvr[:], bv_d[:])
        bpr = pers.tile([1, C], bf16)
        nc.sync.dma_start(bpr[:], bp_d[:])
        b1c = pers.tile([P, MB], f32)
        nc.sync.dma_start(b1c[:], b1_d[:])
        b2r = pers.tile([1, C], bf16)
        nc.sync.dma_start(b2r[:], b2_d[:])
        masks = pers.tile([P, 2, P], bf16)
        nc.sync.dma_start(masks[:], mk_d[:])
        s0c = pers.tile([P, 1], f32)
        nc.sync.dma_start(s0c[:], s0_d[:])
        s1c = pers.tile([P, 1], f32)
        nc.sync.dma_start(s1c[:], s1_d[:])
        eabt = pers.tile([24, 12, P], bf16)
        nc.sync.dma_start(eabt[:], e_d.ap().rearrange("i k p -> k i p"))

        outT_g = [late.tile([P, HP, 512], bf16, tag=f"outT{g}",
                             name=f"outT{g}") for g in range(2)]
        wp_t = late.tile([P, EO, C], bf16)
        h2T_g = [late.tile([P, EO, 512], bf16, tag=f"h2T{g}", name=f"h2T{g}")
                 for g in range(2)]

        KT = attnp.tile([P, HP, T], bf16)
        QT = attnp.tile([P, HP, TQ], bf16)
        V = attnp.tile([P, NB, H, 65], bf16)
        nc.vector.memset(V[:], 1.0)

        def ln_block(x_ap, dst, col, nm, lnp, lns, sq_on_dve=False,
                     warm=None):
            # x_ap: [128, 768] (DRAM or SBUF) -> normalized bf16 rows,
            # transposed into dst[:, :, col:col+128]
            if x_ap.space == bass.MemorySpace.DRAM:
                x_t = lnp.tile([P, C], f32, tag="ln_x", name=f"lnx_{nm}")
                nc.sync.dma_start(x_t[:], x_ap)
            else:
                x_t = x_ap
            s1 = lns.tile([P, 1], f32, tag="ln_s1", name=f"s1_{nm}")
            nc.vector.tensor_reduce(s1[:], x_t[:], mybir.AxisListType.X,
                                    ALU.add)
            sq = lnp.tile([P, C], bf16, tag="ln_sq", name=f"sq_{nm}")
            s2 = lns.tile([P, 1], f32, tag="ln_s2", name=f"s2_{nm}")
            nc.scalar.activation(sq[:], x_t[:], AF.Square, accum_out=s2[:])
            mu = lns.tile([P, 1], f32, tag="ln_mu", name=f"mu_{nm}")
            nc.vector.tensor_scalar_mul(mu[:], s1[:], 1.0 / C)
            mu2 = lns.tile([P, 1], f32, tag="ln_mu2", name=f"mu2_{nm}")
            nc.vector.tensor_scalar(mu2[:], mu[:], mu[:], None, ALU.mult)
            ve = lns.tile([P, 1], f32, tag="ln_ve", name=f"ve_{nm}")
            nc.vector.tensor_scalar(ve[:], s2[:], 1.0 / C, 1e-5,
                                    ALU.mult, ALU.add)
            nc.vector.tensor_scalar(ve[:], ve[:], mu2[:], None, ALU.subtract)
            sd = lns.tile([P, 1], f32, tag="ln_sd", name=f"sd_{nm}")
            nc.scalar.activation(sd[:], ve[:], AF.Sqrt)
            rstd = lns.tile([P, 1], f32, tag="ln_rstd", name=f"rstd_{nm}")
            nc.vector.reciprocal(rstd[:], sd[:])
            nbias = lns.tile([P, 1], f32, tag="ln_nb", name=f"nb_{nm}")
            nc.vector.tensor_scalar(nbias[:], mu[:], rstd[:], -1.0,
                                    ALU.mult, ALU.mult)
            z = lnp.tile([P, C], bf16, tag="ln_z", name=f"z_{nm}")
            nc.scalar.activation(z[:], x_t[:], AF.Identity,
                                 bias=nbias[:], scale=rstd[:])
            nc.sync.dma_start_transpose(dst[:, :, col:col + P], z[:])
            if warm is not None:
                # cheap dependency-chained matmul to keep the PE clock warm
                nc.tensor.matmul(warm[:, 0:512], ones1[:], z[0:1, 0:512],
                                 start=True, stop=True)

        # ---------------- Phase 1+2: LN1 and QKV projections -------------
        w13 = tc.alloc_tile_pool(name="w13", bufs=1)
        lnp = tc.alloc_tile_pool(name="lnp", bufs=3)
        lns = tc.alloc_tile_pool(name="lns", bufs=4)
        pq_ps = tc.alloc_tile_pool(name="pq_ps", bufs=3, space="PSUM")
        pv_ps = tc.alloc_tile_pool(name="pv_ps", bufs=3, space="PSUM")
        warm_ps = tc.alloc_tile_pool(name="warm_ps", bufs=1, space="PSUM")

        wq_t = w13.tile([P, EO, C], bf16)
        nc.sync.dma_start(wq_t[:], wq_d.ap().rearrange("(o p) f -> p o f", p=P))
        wk_t = w13.tile([P, EO, C], bf16)
        nc.sync.dma_start(wk_t[:], wk_d.ap().rearrange("(o p) f -> p o f", p=P))
        wv_t = w13.tile([P, EO, C], bf16)
        nc.sync.dma_start(wv_t[:], wv_d.ap().rearrange("(o p) f -> p o f", p=P))
        hT_g = [w13.tile([P, EO, 512], bf16, tag=f"hT{g}", name=f"hT{g}")
                for g in range(4)]
        hqT_g = [w13.tile([P, EO, 512], bf16, tag=f"hqT{g}", name=f"hqT{g}")
                 for g in range(2)]

        warm1 = warm_ps.tile([P, 512], f32, tag="warm", name="warm1")
        for b in range(NB):
            ln_block(xf_d.ap()[b * P:(b + 1) * P, :], hT_g[b // 4],
                     (b % 4) * P, f"f{b}", lnp, lns, warm=warm1)
        # hqT = parity-selected columns of hT (s0/s1 are 1/0 per core parity)
        hq_tmp = w13.tile([P, EO, 512], bf16)
        for g in range(2):
            for half in range(2):
                blk = hT_g[2 * g + half][:].rearrange(
                    "p o (b two t) -> p o b two t", two=2, t=P)
                dstv = hqT_g[g][:, :, half * 256:(half + 1) * 256].rearrange(
                    "p o (b t) -> p o b t", t=P)
                tmpv = hq_tmp[:, :, half * 256:(half + 1) * 256].rearrange(
                    "p o (b t) -> p o b t", t=P)
                nc.vector.tensor_scalar(tmpv, blk[:, :, :, 0, :], s0c[:],
                                        None, ALU.mult)
                nc.vector.tensor_scalar(dstv, blk[:, :, :, 1, :], s1c[:],
                                        None, ALU.mult)
            nc.vector.tensor_tensor(hqT_g[g][:], hqT_g[g][:],
                                    hq_tmp[:], ALU.add)

        # Q^T (own rows) and K^T (full rows), 2 heads col-packed per tile
        for hp in range(HP):
            for qc in range(2):
                pq = pq_ps.tile([P, 512], f32, tag="pqkt", name=f"pq_{hp}_{qc}")
                for eo in range(EO):
                    for ab in range(2):
                        nc.tensor.matmul(
                            pq[64 * ab:64 * (ab + 1), :],
                            wq_t[:, eo, hp * P + 64 * ab:hp * P + 64 * (ab + 1)],
                            hqT_g[qc][:, eo, :],
                            start=(eo == 0), stop=(eo == EO - 1),
                            tile_position=(0, 64 * ab),
                            skip_group_check=True)
                nc.scalar.activation(QT[:, hp, qc * 512:(qc + 1) * 512],
                                     pq[:], AF.Identity,
                                     bias=bqc[:, hp:hp + 1])
            for cc in range(4):
                pk = pq_ps.tile([P, 512], f32, tag="pqkt", name=f"pk_{hp}_{cc}")
                for eo in range(EO):
                    for ab in range(2):
                        nc.tensor.matmul(
                            pk[64 * ab:64 * (ab + 1), :],
                            wk_t[:, eo, hp * P + 64 * ab:hp * P + 64 * (ab + 1)],
                            hT_g[cc][:, eo, :],
                            start=(eo == 0), stop=(eo == EO - 1),
                            tile_position=(0, 64 * ab),
                            skip_group_check=True)
                nc.scalar.activation(KT[:, hp, cc * 512:(cc + 1) * 512],
                                     pk[:], AF.Identity,
                                     bias=bkc[:, hp:hp + 1])

        # V natural [c, f] with a ones column at f=64 per head
        for cb in range(NB):
            for fo in range(2):
                pv = pv_ps.tile([P, 384], f32, tag="pv", name=f"pv_{cb}_{fo}")
                for eo in range(EO):
                    nc.tensor.matmul(
                        pv[:], hT_g[cb // 4][:, eo, (cb % 4) * P:(cb % 4 + 1) * P],
                        wv_t[:, eo, fo * 384:(fo + 1) * 384],
                        start=(eo == 0), stop=False)
                nc.tensor.matmul(pv[:], ones1[:],
                                 bvr[:, fo * 384:(fo + 1) * 384],
                                 start=False, stop=True)
                nc.scalar.activation(
                    V[:, cb, fo * 6:(fo + 1) * 6, 0:64],
                    pv[:].rearrange("p (g d) -> p g d", g=6),
                    AF.Identity)

        for _pool in (warm_ps, pv_ps, pq_ps, lns, lnp, w13):
            _pool.release()

        # prefetch the projection weight while attention runs
        nc.sync.dma_start(wp_t[:], wp_d.ap().rearrange("(o p) f -> p o f", p=P))

        # ---------------- Phase 3: attention -----------------------------
        pr = tc.alloc_tile_pool(name="pr", bufs=3)
        rsn = tc.alloc_tile_pool(name="rsn", bufs=2)
        psc_ps = tc.alloc_tile_pool(name="psc", bufs=2, space="PSUM")
        pso_ps = tc.alloc_tile_pool(name="pso", bufs=2, space="PSUM")

        # rs16_pad rows 0:24 hold bf16 row-sums; the rest are 1.0 filler so
        # the padded reciprocal stays finite.
        rs16_pad = rsn.tile([P, 512], bf16, name="rs16_pad")
        nc.vector.memset(rs16_pad[:], 1.0)

        for hp in range(HP):
            for qc in range(2):
                q0 = qc * 512
                poA = pso_ps.tile([P, 512], f32, tag="poA", name=f"poA_{hp}_{qc}")
                poB = pso_ps.tile([P, 512], f32, tag="poB", name=f"poB_{hp}_{qc}")
                po_t = [poA, poB]
                jmax = 8 if qc == 0 else 16
                for j in range(jmax):
                    qsj = (j // 2) * P
                    qs = max(qsj, q0)
                    off = qs - q0
                    N = 512 - off
                    psc = psc_ps.tile([P, 2, 512], f32, tag="psc",
                                      name=f"psc_{hp}_{qc}_{j}")
                    for ab in range(2):
                        nc.tensor.matmul(
                            psc[:, ab, off:off + N],
                            KT[64 * ab:64 * (ab + 1), hp, j * P:(j + 1) * P],
                            QT[64 * ab:64 * (ab + 1), hp, qs:qs + N],
                            start=True, stop=True,
                            tile_position=(64 * ab, 0))
                    probs = pr.tile([P, 2, 512], bf16, tag="probs",
                                    name=f"pb_{hp}_{qc}_{j}")
                    nc.scalar.activation(probs[:, :, off:off + N],
                                         psc[:, :, off:off + N],
                                         AF.Exp, scale=0.125)
                    if qs == qsj:
                        nc.vector.tensor_tensor(
                            probs[:, :, off:off + P],
                            probs[:, :, off:off + P],
                            masks[:, j % 2, None, :].to_broadcast(
                                (P, 2, P)), ALU.mult)
                    for ab in range(2):
                        nc.tensor.matmul(
                            po_t[ab][0:65, off:off + N],
                            V[:, j, 2 * hp + ab, :],
                            probs[:, ab, off:off + N],
                            start=(j == 0), stop=(j == jmax - 1))
                # evict unnormalized out^T and stash the row-sums (row 64)
                for ab in range(2):
                    nc.vector.tensor_copy(
                        out=outT_g[qc][64 * ab:64 * (ab + 1), hp, :],
                        in_=po_t[ab][0:64, :])
                    rstmp = rsn.tile([1, 512], bf16, tag="rstmp",
                                     name=f"rst_{hp}_{qc}_{ab}")
                    nc.vector.tensor_copy(out=rstmp[:],
                                          in_=po_t[ab][64:65, :])
                    k = hp * 4 + qc * 2 + ab
                    nc.sync.dma_start(rs16_pad[k:k + 1, :], rstmp[:])

        for _pool in (pso_ps, psc_ps):
            _pool.release()

        # ---- deferred normalization of outT ------------------------------
        nrm_ps = tc.alloc_tile_pool(name="nrm", bufs=2, space="PSUM")
        rsT = rsn.tile([P, 4, P], bf16, name="rsT")
        for c in range(4):
            nc.scalar.dma_start_transpose(rsT[:, c, :],
                                          rs16_pad[:, c * P:(c + 1) * P])
        rsTf = rsn.tile([P, 4 * P], f32, name="rsTf")
        nc.vector.tensor_copy(out=rsTf[:], in_=rsT[:])
        recTf = rsn.tile([P, 4 * P], f32, name="recTf")
        nc.vector.reciprocal(recTf[:], rsTf[:])
        recT16 = rsn.tile([P, 4, P], bf16, name="recT16")
        nc.vector.tensor_copy(out=recT16[:], in_=recTf[:])
        rec16_pad = rsn.tile([P, 512], bf16, name="rec16_pad")
        for c in range(4):
            nc.scalar.dma_start_transpose(rec16_pad[:, c * P:(c + 1) * P],
                                          recT16[:, c, :])
        for qc in range(2):
            for hp in range(HP):
                pb = nrm_ps.tile([P, 512], f32, tag="pbn",
                                 name=f"pbn_{hp}_{qc}")
                nc.tensor.matmul(pb[:], eabt[:, hp * 2 + qc, :],
                                 rec16_pad[0:24, :],
                                 start=True, stop=True)
                nc.vector.tensor_tensor(
                    outT_g[qc][:, hp, :], outT_g[qc][:, hp, :],
                    pb[:], ALU.mult)

        for _pool in (nrm_ps, rsn, pr):
            _pool.release()
        attnp.release()

        # -------- Phase 4: projection + residual + LN2 --------------------
        # FFN weights prefetch during proj (pool allocated below the proj
        # transients so it survives into the FFN phase)
        ffn = tc.alloc_tile_pool(name="ffn", bufs=1)
        w1_t = ffn.tile([P, EO, FF], bf16)
        for eo in range(EO):
            nc.gpsimd.dma_start(
                w1_t[:, eo, :], w1_d.ap()[eo * P:(eo + 1) * P, :])
        w2_t = ffn.tile([P, MB, C], bf16)
        for c in range(4):
            nc.gpsimd.dma_start(
                w2_t[:, 6 * c:6 * (c + 1), :],
                w2_d.ap()[c * C:(c + 1) * C, :].rearrange(
                    "(o p) f -> p o f", p=P))
        uT = ffn.tile([P, MB, TQ], bf16)

        lnp2 = tc.alloc_tile_pool(name="lnp2", bufs=3)
        lns2 = tc.alloc_tile_pool(name="lns2", bufs=4)
        xres = tc.alloc_tile_pool(name="xres", bufs=3)
        ppr_ps = tc.alloc_tile_pool(name="ppr", bufs=2, space="PSUM")
        warm2_ps = tc.alloc_tile_pool(name="warm2_ps", bufs=1, space="PSUM")

        warm2 = warm2_ps.tile([P, 512], f32, tag="warm2", name="warm2")
        for qb in range(NQ):
            xqb = xres.tile([P, C], f32, tag="xqb", name=f"xqb_{qb}")
            nc.sync.dma_start(xqb[:], xq_d.ap()[qb * P:(qb + 1) * P, :])
            x2b = xres.tile([P, C], f32, tag="x2b", name=f"x2b_{qb}")
            for fo in range(2):
                pp = ppr_ps.tile([P, 384], f32, tag="ppr", name=f"pp_{qb}_{fo}")
                for fp in range(EO):
                    nc.tensor.matmul(
                        pp[:],
                        outT_g[qb // 4][:, fp, (qb % 4) * P:(qb % 4 + 1) * P],
                        wp_t[:, fp, fo * 384:(fo + 1) * 384],
                        start=(fp == 0), stop=False)
                nc.tensor.matmul(pp[:], ones1[:],
                                 bpr[:, fo * 384:(fo + 1) * 384],
                                 start=False, stop=True)
                nc.vector.tensor_tensor(
                    x2b[:, fo * 384:(fo + 1) * 384], pp[:],
                    xqb[:, fo * 384:(fo + 1) * 384], ALU.add)
            nc.sync.dma_start(x2_d.ap()[:, qb, :], x2b[:])
            ln_block(x2b[:], h2T_g[qb // 4], (qb % 4) * P,
                     f"x2{qb}", lnp2, lns2, sq_on_dve=True,
                     warm=warm2)

        for _pool in (warm2_ps, ppr_ps, xres, lns2, lnp2):
            _pool.release()

        # -------- Phase 5: FFN --------------------------------------------
        oup = tc.alloc_tile_pool(name="oup", bufs=3)
        pu_ps = tc.alloc_tile_pool(name="pu", bufs=2, space="PSUM")
        py_ps = tc.alloc_tile_pool(name="py", bufs=2, space="PSUM")

        for mb in range(MB):
            for qc2 in range(2):
                pu = pu_ps.tile([P, 512], f32, tag="pu", name=f"pu_{mb}_{qc2}")
                for eo in range(EO):
                    nc.tensor.matmul(
                        pu[:], w1_t[:, eo, mb * P:(mb + 1) * P],
                        h2T_g[qc2][:, eo, :],
                        start=(eo == 0), stop=(eo == EO - 1))
                nc.vector.tensor_scalar(
                    uT[:, mb, qc2 * 512:(qc2 + 1) * 512], pu[:],
                    b1c[:, mb:mb + 1], 0.0, ALU.add, ALU.max)
        for qb in range(NQ):
            x2r = oup.tile([P, C], f32, tag="x2r", name=f"x2r_{qb}")
            nc.sync.dma_start(x2r[:], x2_d.ap()[:, qb, :])
            for fo in range(2):
                py = py_ps.tile([P, 384], f32, tag="py", name=f"py_{qb}_{fo}")
                for mp in range(MB):
                    nc.tensor.matmul(
                        py[:], uT[:, mp, qb * P:(qb + 1) * P],
                        w2_t[:, mp, fo * 384:(fo + 1) * 384],
                        start=(mp == 0), stop=False)
                nc.tensor.matmul(py[:], ones1[:],
                                 b2r[:, fo * 384:(fo + 1) * 384],
                                 start=False, stop=True)
                ot = oup.tile([P, 384], f32, tag="ot", name=f"ot_{qb}_{fo}")
                nc.vector.tensor_tensor(
                    ot[:], py[:], x2r[:, fo * 384:(fo + 1) * 384], ALU.add)
                nc.sync.dma_start(
                    out_d.ap()[qb * P:(qb + 1) * P,
                               fo * 384:(fo + 1) * 384], ot[:])

        for _pool in (py_ps, pu_ps, oup, ffn, late, pers):
            _pool.release()

    return nc


def prepare_in_maps(inputs):
    """Build the 8 per-core input maps from the full problem inputs."""
    x = np.asarray(inputs["x"], np.float32)
    wq = np.asarray(inputs["wq"], np.float32)
    wk = np.asarray(inputs["wk"], np.float32)
    wv = np.asarray(inputs["wv"], np.float32)
    w_proj = np.asarray(inputs["w_proj"], np.float32)
    b_proj = np.asarray(inputs["b_proj"], np.float32)
    w1 = np.asarray(inputs["w1"], np.float32)
    b1 = np.asarray(inputs["b1"], np.float32)
    w2 = np.asarray(inputs["w2"], np.float32)
    b2 = np.asarray(inputs["b2"], np.float32)
    g1 = np.asarray(inputs["ln1_g"], np.float32)
    be1 = np.asarray(inputs["ln1_b"], np.float32)
    g2 = np.asarray(inputs["ln2_g"], np.float32)
    be2 = np.asarray(inputs["ln2_b"], np.float32)

    bf = ml_dtypes.bfloat16
    wq_r = wq.transpose(1, 0, 2).reshape(C, C)       # [c, h*d]
    wk_r = wk.transpose(1, 0, 2).reshape(C, C)
    wv_r = wv.transpose(1, 0, 2).reshape(C, C)
    wq_g = (g1[:, None] * wq_r).astype(bf)
    wk_g = (g1[:, None] * wk_r).astype(bf)
    wv_g = (g1[:, None] * wv_r).astype(bf)
    bq = (be1 @ wq_r).reshape(HP, P).T.copy().astype(np.float32)   # [128, hp]
    bk = (be1 @ wk_r).reshape(HP, P).T.copy().astype(np.float32)
    bv = (be1 @ wv_r).reshape(1, C).astype(bf)
    w1_g = (g2[:, None] * w1).astype(bf)
    b1f = (b1 + be2 @ w1).reshape(MB, P).T.copy().astype(np.float32)  # [128, mb]
    wp16 = w_proj.astype(bf)
    w2_16 = w2.astype(bf)
    bp = b_proj.reshape(1, C).astype(bf)
    b2r = b2.reshape(1, C).astype(bf)

    ci = np.arange(P)[:, None]
    qi = np.arange(P)[None, :]
    tri = (ci <= qi).astype(np.float32)          # visible where c <= q
    m_par = [
        np.stack([tri, np.zeros((P, P), np.float32)], 0),   # parity 0
        np.stack([np.ones((P, P), np.float32), tri], 0),    # parity 1
    ]

    eab = np.zeros((12, 24, P), np.float32)
    for i in range(12):
        hp_, qc_ = i // 2, i % 2
        eab[i, hp_ * 4 + qc_ * 2 + 0, 0:64] = 1.0
        eab[i, hp_ * 4 + qc_ * 2 + 1, 64:128] = 1.0
    eab16 = eab.astype(bf)

    in_maps = []
    for core in range(8):
        b, p = core // 2, core % 2
        xf = np.ascontiguousarray(x[b])
        xq = np.ascontiguousarray(
            x[b].reshape(NB, P, C)[p::2].reshape(TQ, C))
        mk = np.ascontiguousarray(m_par[p].transpose(1, 0, 2)).astype(bf)
        s0 = np.full((P, 1), 1.0 - p, np.float32)
        s1 = np.full((P, 1), float(p), np.float32)
        in_maps.append({
            "xq": xq, "xf": xf,
            "wq": wq_g, "wk": wk_g, "wv": wv_g, "wp": wp16,
            "w1": w1_g, "w2": w2_16,
            "bq": bq, "bk": bk, "bv": bv, "bp": bp, "b1": b1f, "b2": b2r,
            "mk": mk, "s0": s0, "s1": s1, "eab": eab16,
        })
    return in_maps


def assemble_output(results):
    """Reassemble the 8 per-core [1024, 768] outputs into [4, 2048, 768]."""
    out = np.empty((4, T, C), np.float32)
    for core in range(8):
        b, p = core // 2, core % 2
        blocks = results[core]["out"].reshape(NQ, P, C)
        ov = out[b].reshape(NB, P, C)
        ov[p::2] = blocks
    return out


_CACHED_NC = None


def kernel(**inputs) -> np.ndarray:
    global _CACHED_NC
    from concourse.bass_utils import run_bass_kernel_spmd

    if _CACHED_NC is None:
        nc = build_program()
        split_multiwait_instructions(nc)
        _CACHED_NC = nc
    in_maps = prepare_in_maps(inputs)
    res = run_bass_kernel_spmd(_CACHED_NC, in_maps, list(range(8)))
    return assemble_output(res.results)


# revision 21
# speedup vs baseline: 1.0275x; 1.0102x over previous
"""Trainium2 Bass kernel for a dense transformer block (attention + FFN).

Sharding: data-parallel over (batch, sequence-parity). 8 cores = 4 batches x 2
parity groups. Core c handles batch b = c//2 and the 128-row blocks of parity
p = c%2 (blocks p, p+2, ..., p+14) as query rows; K/V are computed for the
full sequence of the batch on-core (no collectives). The causal structure is
made SPMD-uniform by computing, for query block i, key blocks j <= 2i+1 and
masking with a small per-core multiplicative mask input.

On-chip layout: activations are kept feature-major (transposed) where matmuls
need them as stationary/moving operands; scores are computed transposed
(S^T[c,q]) so softmax probabilities feed the attention*V matmul directly with
no transpose, and the row-sum comes for free from an appended ones-column in
the V stationary operand.
"""
import sys

sys.path.insert(0, '/opt/trn_rl_repo')

import numpy as np
import ml_dtypes

import bass_rust
import concourse.bass as bass
import concourse.tile as tile
from concourse import mybir

P = 128
T = 2048
TQ = 1024
C = 768
H = 12
D = 64
FF = 3072
EO = C // P          # 6
MB = FF // P         # 24
NB = T // P          # 16
NQ = TQ // P         # 8
HP = H // 2          # 6

f32 = mybir.dt.float32
bf16 = mybir.dt.bfloat16
AF = mybir.ActivationFunctionType
ALU = mybir.AluOpType


def split_multiwait_instructions(nc):
    """The installed walrus build rejects any instruction carrying more than
    one sync wait; hoist extra waits onto NoOps inserted before it on the
    same (serial) engine."""
    n_fixed = 0
    for f in nc.m.functions:
        for bb in f.blocks:
            insts = bb.instructions
            new_insts = []
            dirty = False
            for inst in insts:
                si = inst.sync_info
                waits = list(si.on_wait) if si and si.on_wait else []
                if len(waits) > 1:
                    for j, w in enumerate(waits[:-1]):
                        nop = bass_rust.InstNoOp(
                            name=f"{inst.name}_sw{j}", ins=[], outs=[]
                        )
                        nop.engine = inst.engine
                        nop.sync_info = bass_rust.SyncInfo(
                            on_wait=[w], on_update=[]
                        )
                        new_insts.append(nop)
                    si.on_wait = waits[-1:]
                    dirty = True
                    n_fixed += 1
                new_insts.append(inst)
            if dirty:
                bb.instructions = new_insts
    return n_fixed


def build_program():
    """Build the single SPMD program (identical on all 8 cores)."""
    nc = bass.Bass("TRN2", target_bir_lowering=False, debug=False,
                   num_devices=8)

    xq_d = nc.declare_dram_parameter("xq", [TQ, C], f32, isOutput=False)
    xf_d = nc.declare_dram_parameter("xf", [T, C], f32, isOutput=False)
    wq_d = nc.declare_dram_parameter("wq", [C, C], bf16, isOutput=False)
    wk_d = nc.declare_dram_parameter("wk", [C, C], bf16, isOutput=False)
    wv_d = nc.declare_dram_parameter("wv", [C, C], bf16, isOutput=False)
    wp_d = nc.declare_dram_parameter("wp", [C, C], bf16, isOutput=False)
    w1_d = nc.declare_dram_parameter("w1", [C, FF], bf16, isOutput=False)
    w2_d = nc.declare_dram_parameter("w2", [FF, C], bf16, isOutput=False)
    bq_d = nc.declare_dram_parameter("bq", [P, HP], f32, isOutput=False)
    bk_d = nc.declare_dram_parameter("bk", [P, HP], f32, isOutput=False)
    bv_d = nc.declare_dram_parameter("bv", [1, C], bf16, isOutput=False)
    bp_d = nc.declare_dram_parameter("bp", [1, C], bf16, isOutput=False)
    b1_d = nc.declare_dram_parameter("b1", [P, MB], f32, isOutput=False)
    b2_d = nc.declare_dram_parameter("b2", [1, C], bf16, isOutput=False)
    mk_d = nc.declare_dram_parameter("mk", [P, 2, P], bf16, isOutput=False)
    s0_d = nc.declare_dram_parameter("s0", [P, 1], f32, isOutput=False)
    s1_d = nc.declare_dram_parameter("s1", [P, 1], f32, isOutput=False)
    e_d = nc.declare_dram_parameter("eab", [12, 24, P], bf16, isOutput=False)
    out_d = nc.declare_dram_parameter("out", [TQ, C], f32, isOutput=True)

    x2_d = nc.dram_tensor("x2scratch", [P, NQ, C], f32)

    with tile.TileContext(nc) as tc:
        # --- pool stack (released LIFO) -------------------------------
        pers = tc.alloc_tile_pool(name="pers", bufs=1)
        late = tc.alloc_tile_pool(name="late", bufs=1)   # outT, wp, h2T
        attnp = tc.alloc_tile_pool(name="attnp", bufs=1)  # KT, QT, V

        ones1 = pers.tile([1, P], bf16)
        nc.vector.memset(ones1[:], 1.0)
        ones64 = pers.tile([1, 64], bf16)
        nc.vector.memset(ones64[:], 1.0)
        bqc = pers.tile([P, HP], f32)
        nc.sync.dma_start(bqc[:], bq_d[:])
        bkc = pers.tile([P, HP], f32)
        nc.sync.dma_start(bkc[:], bk_d[:])
        bvr = pers.tile([1, C], bf16)
        nc.sync.dma_start(bvr[:], bv_d[:])
        bpr = pers.tile([1, C], bf16)
        nc.sync.dma_start(bpr[:], bp_d[:])
        b1c = pers.tile([P, MB], f32)
        nc.sync.dma_start(b1c[:], b1_d[:])
        b2r = pers.tile([1, C], bf16)
        nc.sync.dma_start(b2r[:], b2_d[:])
        masks = pers.tile([P, 2, P], bf16)
        nc.sync.dma_start(masks[:], mk_d[:])
        s0c = pers.tile([P, 1], f32)
        nc.sync.dma_start(s0c[:], s0_d[:])
        s1c = pers.tile([P, 1], f32)
        nc.sync.dma_start(s1c[:], s1_d[:])
        eabt = pers.tile([24, 12, P], bf16)
        nc.sync.dma_start(eabt[:], e_d.ap().rearrange("i k p -> k i p"))

        outT_g = [late.tile([P, HP, 512], bf16, tag=f"outT{g}",
                             name=f"outT{g}") for g in range(2)]
        wp_t = late.tile([P, EO, C], bf16)
        h2T_g = [late.tile([P, EO, 512], bf16, tag=f"h2T{g}", name=f"h2T{g}")
                 for g in range(2)]

        KT = attnp.tile([P, HP, T], bf16)
        QT = attnp.tile([P, HP, TQ], bf16)
        V = attnp.tile([P, NB, H, 65], bf16)
        nc.vector.memset(V[:], 1.0)

        def ln_block(x_ap, dst, col, nm, lnp, lns, sq_on_dve=False,
                     warm=None):
            # x_ap: [128, 768] (DRAM or SBUF) -> normalized bf16 rows,
            # transposed into dst[:, :, col:col+128]
            if x_ap.space == bass.MemorySpace.DRAM:
                x_t = lnp.tile([P, C], f32, tag="ln_x", name=f"lnx_{nm}")
                nc.sync.dma_start(x_t[:], x_ap)
            else:
                x_t = x_ap
            s1 = lns.tile([P, 1], f32, tag="ln_s1", name=f"s1_{nm}")
            nc.vector.tensor_reduce(s1[:], x_t[:], mybir.AxisListType.X,
                                    ALU.add)
            sq = lnp.tile([P, C], bf16, tag="ln_sq", name=f"sq_{nm}")
            s2 = lns.tile([P, 1], f32, tag="ln_s2", name=f"s2_{nm}")
            nc.scalar.activation(sq[:], x_t[:], AF.Square, accum_out=s2[:])
            mu = lns.tile([P, 1], f32, tag="ln_mu", name=f"mu_{nm}")
            nc.vector.tensor_scalar_mul(mu[:], s1[:], 1.0 / C)
            mu2 = lns.tile([P, 1], f32, tag="ln_mu2", name=f"mu2_{nm}")
            nc.vector.tensor_scalar(mu2[:], mu[:], mu[:], None, ALU.mult)
            ve = lns.tile([P, 1], f32, tag="ln_ve", name=f"ve_{nm}")
            nc.vector.tensor_scalar(ve[:], s2[:], 1.0 / C, 1e-5,
                                    ALU.mult, ALU.add)
            nc.vector.tensor_scalar(ve[:], ve[:], mu2[:], None, ALU.subtract)
            sd = lns.tile([P, 1], f32, tag="ln_sd", name=f"sd_{nm}")
            nc.scalar.activation(sd[:], ve[:], AF.Sqrt)
            rstd = lns.tile([P, 1], f32, tag="ln_rstd", name=f"rstd_{nm}")
            nc.vector.reciprocal(rstd[:], sd[:])
            nbias = lns.tile([P, 1], f32, tag="ln_nb", name=f"nb_{nm}")
            nc.vector.tensor_scalar(nbias[:], mu[:], rstd[:], -1.0,
                                    ALU.mult, ALU.mult)
            z = lnp.tile([P, C], bf16, tag="ln_z", name=f"z_{nm}")
            nc.scalar.activation(z[:], x_t[:], AF.Identity,
                                 bias=nbias[:], scale=rstd[:])
            nc.sync.dma_start_transpose(dst[:, :, col:col + P], z[:])
            if warm is not None:
                # cheap dependency-chained matmul to keep the PE clock warm
                nc.tensor.matmul(warm[:, 0:512], ones1[:], z[0:1, 0:512],
                                 start=True, stop=True)

        # ---------------- Phase 1+2: LN1 and QKV projections -------------
        w13 = tc.alloc_tile_pool(name="w13", bufs=1)
        lnp = tc.alloc_tile_pool(name="lnp", bufs=3)
        lns = tc.alloc_tile_pool(name="lns", bufs=4)
        pq_ps = tc.alloc_tile_pool(name="pq_ps", bufs=3, space="PSUM")
        pv_ps = tc.alloc_tile_pool(name="pv_ps", bufs=3, space="PSUM")
        warm_ps = tc.alloc_tile_pool(name="warm_ps", bufs=1, space="PSUM")

        wq_t = w13.tile([P, EO, C], bf16)
        nc.sync.dma_start(wq_t[:], wq_d.ap().rearrange("(o p) f -> p o f", p=P))
        wk_t = w13.tile([P, EO, C], bf16)
        nc.sync.dma_start(wk_t[:], wk_d.ap().rearrange("(o p) f -> p o f", p=P))
        wv_t = w13.tile([P, EO, C], bf16)
        nc.sync.dma_start(wv_t[:], wv_d.ap().rearrange("(o p) f -> p o f", p=P))
        hT_g = [w13.tile([P, EO, 512], bf16, tag=f"hT{g}", name=f"hT{g}")
                for g in range(4)]
        hqT_g = [w13.tile([P, EO, 512], bf16, tag=f"hqT{g}", name=f"hqT{g}")
                 for g in range(2)]

        warm1 = warm_ps.tile([P, 512], f32, tag="warm", name="warm1")
        for b in range(NB):
            ln_block(xf_d.ap()[b * P:(b + 1) * P, :], hT_g[b // 4],
                     (b % 4) * P, f"f{b}", lnp, lns, warm=warm1)
        # hqT = parity-selected columns of hT (s0/s1 are 1/0 per core parity)
        hq_tmp = w13.tile([P, EO, 512], bf16)
        for g in range(2):
            for half in range(2):
                blk = hT_g[2 * g + half][:].rearrange(
                    "p o (b two t) -> p o b two t", two=2, t=P)
                dstv = hqT_g[g][:, :, half * 256:(half + 1) * 256].rearrange(
                    "p o (b t) -> p o b t", t=P)
                tmpv = hq_tmp[:, :, half * 256:(half + 1) * 256].rearrange(
                    "p o (b t) -> p o b t", t=P)
                nc.vector.tensor_scalar(tmpv, blk[:, :, :, 0, :], s0c[:],
                                        None, ALU.mult)
                nc.vector.tensor_scalar(dstv, blk[:, :, :, 1, :], s1c[:],
                                        None, ALU.mult)
            nc.vector.tensor_tensor(hqT_g[g][:], hqT_g[g][:],
                                    hq_tmp[:], ALU.add)

        # Q^T (own rows) and K^T (full rows), 2 heads col-packed per tile
        for hp in range(HP):
            for qc in range(2):
                pq = pq_ps.tile([P, 512], f32, tag="pqkt", name=f"pq_{hp}_{qc}")
                for eo in range(EO):
                    for ab in range(2):
                        nc.tensor.matmul(
                            pq[64 * ab:64 * (ab + 1), :],
                            wq_t[:, eo, hp * P + 64 * ab:hp * P + 64 * (ab + 1)],
                            hqT_g[qc][:, eo, :],
                            start=(eo == 0), stop=(eo == EO - 1),
                            tile_position=(0, 64 * ab),
                            skip_group_check=True)
                nc.scalar.activation(QT[:, hp, qc * 512:(qc + 1) * 512],
                                     pq[:], AF.Identity,
                                     bias=bqc[:, hp:hp + 1])
            for cc in range(4):
                pk = pq_ps.tile([P, 512], f32, tag="pqkt", name=f"pk_{hp}_{cc}")
                for eo in range(EO):
                    for ab in range(2):
                        nc.tensor.matmul(
                            pk[64 * ab:64 * (ab + 1), :],
                            wk_t[:, eo, hp * P + 64 * ab:hp * P + 64 * (ab + 1)],
                            hT_g[cc][:, eo, :],
                            start=(eo == 0), stop=(eo == EO - 1),
                            tile_position=(0, 64 * ab),
                            skip_group_check=True)
                nc.scalar.activation(KT[:, hp, cc * 512:(cc + 1) * 512],
                                     pk[:], AF.Identity,
                                     bias=bkc[:, hp:hp + 1])

        # V natural [c, f] with a ones column at f=64 per head
        for cb in range(NB):
            for fo in range(2):
                pv = pv_ps.tile([P, 384], f32, tag="pv", name=f"pv_{cb}_{fo}")
                for eo in range(EO):
                    nc.tensor.matmul(
                        pv[:], hT_g[cb // 4][:, eo, (cb % 4) * P:(cb % 4 + 1) * P],
                        wv_t[:, eo, fo * 384:(fo + 1) * 384],
                        start=(eo == 0), stop=False)
                nc.tensor.matmul(pv[:], ones1[:],
                                 bvr[:, fo * 384:(fo + 1) * 384],
                                 start=False, stop=True)
                nc.scalar.activation(
                    V[:, cb, fo * 6:(fo + 1) * 6, 0:64],
                    pv[:].rearrange("p (g d) -> p g d", g=6),
                    AF.Identity)

        for _pool in (warm_ps, pv_ps, pq_ps, lns, lnp, w13):
            _pool.release()

        # prefetch the projection weight while attention runs
        nc.sync.dma_start(wp_t[:], wp_d.ap().rearrange("(o p) f -> p o f", p=P))

        # ---------------- Phase 3: attention -----------------------------
        pr = tc.alloc_tile_pool(name="pr", bufs=3)
        rsn = tc.alloc_tile_pool(name="rsn", bufs=2)
        psc_ps = tc.alloc_tile_pool(name="psc", bufs=2, space="PSUM")
        pso_ps = tc.alloc_tile_pool(name="pso", bufs=2, space="PSUM")

        # rs16_pad rows 0:24 hold bf16 row-sums; the rest are 1.0 filler so
        # the padded reciprocal stays finite.
        rs16_pad = rsn.tile([P, 512], bf16, name="rs16_pad")
        nc.vector.memset(rs16_pad[:], 1.0)

        for hp in range(HP):
            for qc in range(2):
                q0 = qc * 512
                poA = pso_ps.tile([P, 512], f32, tag="poA", name=f"poA_{hp}_{qc}")
                poB = pso_ps.tile([P, 512], f32, tag="poB", name=f"poB_{hp}_{qc}")
                po_t = [poA, poB]
                jmax = 8 if qc == 0 else 16
                for j in range(jmax):
                    qsj = (j // 2) * P
                    qs = max(qsj, q0)
                    off = qs - q0
                    N = 512 - off
                    psc = psc_ps.tile([P, 2, 512], f32, tag="psc",
                                      name=f"psc_{hp}_{qc}_{j}")
                    for ab in range(2):
                        nc.tensor.matmul(
                            psc[:, ab, off:off + N],
                            KT[64 * ab:64 * (ab + 1), hp, j * P:(j + 1) * P],
                            QT[64 * ab:64 * (ab + 1), hp, qs:qs + N],
                            start=True, stop=True,
                            tile_position=(64 * ab, 0))
                    probs = pr.tile([P, 2, 512], bf16, tag="probs",
                                    name=f"pb_{hp}_{qc}_{j}")
                    nc.scalar.activation(probs[:, :, off:off + N],
                                         psc[:, :, off:off + N],
                                         AF.Exp, scale=0.125)
                    if qs == qsj:
                        nc.vector.tensor_tensor(
                            probs[:, :, off:off + P],
                            probs[:, :, off:off + P],
                            masks[:, j % 2, None, :].to_broadcast(
                                (P, 2, P)), ALU.mult)
                    for ab in range(2):
                        nc.tensor.matmul(
                            po_t[ab][0:65, off:off + N],
                            V[:, j, 2 * hp + ab, :],
                            probs[:, ab, off:off + N],
                            start=(j == 0), stop=(j == jmax - 1))
                # evict unnormalized out^T and stash the row-sums (row 64)
                for ab in range(2):
                    nc.vector.tensor_copy(
                        out=outT_g[qc][64 * ab:64 * (ab + 1), hp, :],
                        in_=po_t[ab][0:64, :])
                    rstmp = rsn.tile([1, 512], bf16, tag="rstmp",
                                     name=f"rst_{hp}_{qc}_{ab}")
                    nc.vector.tensor_copy(out=rstmp[:],
                                          in_=po_t[ab][64:65, :])
                    k = hp * 4 + qc * 2 + ab
                    nc.sync.dma_start(rs16_pad[k:k + 1, :], rstmp[:])

        for _pool in (pso_ps, psc_ps):
            _pool.release()

        # ---- deferred normalization of outT ------------------------------
        nrm_ps = tc.alloc_tile_pool(name="nrm", bufs=2, space="PSUM")
        rsT = rsn.tile([P, 4, P], bf16, name="rsT")
        for c in range(4):
            nc.scalar.dma_start_transpose(rsT[:, c, :],
                                          rs16_pad[:, c * P:(c + 1) * P])
        rsTf = rsn.tile([P, 4 * P], f32, name="rsTf")
        nc.vector.tensor_copy(out=rsTf[:], in_=rsT[:])
        recTf = rsn.tile([P, 4 * P], f32, name="recTf")
        nc.vector.reciprocal(recTf[:], rsTf[:])
        recT16 = rsn.tile([P, 4, P], bf16, name="recT16")
        nc.vector.tensor_copy(out=recT16[:], in_=recTf[:])
        rec16_pad = rsn.tile([P, 512], bf16, name="rec16_pad")
        for c in range(4):
            nc.scalar.dma_start_transpose(rec16_pad[:, c * P:(c + 1) * P],
                                          recT16[:, c, :])
        for qc in range(2):
            for hp in range(HP):
                pb = nrm_ps.tile([P, 512], f32, tag="pbn",
                                 name=f"pbn_{hp}_{qc}")
                nc.tensor.matmul(pb[:], eabt[:, hp * 2 + qc, :],
                                 rec16_pad[0:24, :],
                                 start=True, stop=True)
                nc.vector.tensor_tensor(
                    outT_g[qc][:, hp, :], outT_g[qc][:, hp, :],
                    pb[:], ALU.mult)

        for _pool in (nrm_ps, rsn, pr):
            _pool.release()
        attnp.release()

        # -------- Phase 4: projection + residual + LN2 --------------------
        # FFN weights prefetch during proj (pool allocated below the proj
        # transients so it survives into the FFN phase)
        ffn = tc.alloc_tile_pool(name="ffn", bufs=1)
        w1_t = ffn.tile([P, EO, FF], bf16)
        for eo in range(EO):
            nc.gpsimd.dma_start(
                w1_t[:, eo, :], w1_d.ap()[eo * P:(eo + 1) * P, :])
        w2_t = ffn.tile([P, MB, C], bf16)
        for mp in range(MB):
            nc.gpsimd.dma_start(
                w2_t[:, mp, :], w2_d.ap()[mp * P:(mp + 1) * P, :])
        uT = ffn.tile([P, MB, TQ], bf16)

        lnp2 = tc.alloc_tile_pool(name="lnp2", bufs=3)
        lns2 = tc.alloc_tile_pool(name="lns2", bufs=4)
        xres = tc.alloc_tile_pool(name="xres", bufs=3)
        ppr_ps = tc.alloc_tile_pool(name="ppr", bufs=2, space="PSUM")
        warm2_ps = tc.alloc_tile_pool(name="warm2_ps", bufs=1, space="PSUM")

        warm2 = warm2_ps.tile([P, 512], f32, tag="warm2", name="warm2")
        for qb in range(NQ):
            xqb = xres.tile([P, C], f32, tag="xqb", name=f"xqb_{qb}")
            nc.sync.dma_start(xqb[:], xq_d.ap()[qb * P:(qb + 1) * P, :])
            x2b = xres.tile([P, C], f32, tag="x2b", name=f"x2b_{qb}")
            for fo in range(2):
                pp = ppr_ps.tile([P, 384], f32, tag="ppr", name=f"pp_{qb}_{fo}")
                for fp in range(EO):
                    nc.tensor.matmul(
                        pp[:],
                        outT_g[qb // 4][:, fp, (qb % 4) * P:(qb % 4 + 1) * P],
                        wp_t[:, fp, fo * 384:(fo + 1) * 384],
                        start=(fp == 0), stop=False)
                nc.tensor.matmul(pp[:], ones1[:],
                                 bpr[:, fo * 384:(fo + 1) * 384],
                                 start=False, stop=True)
                nc.vector.tensor_tensor(
                    x2b[:, fo * 384:(fo + 1) * 384], pp[:],
                    xqb[:, fo * 384:(fo + 1) * 384], ALU.add)
            nc.sync.dma_start(x2_d.ap()[:, qb, :], x2b[:])
            ln_block(x2b[:], h2T_g[qb // 4], (qb % 4) * P,
                     f"x2{qb}", lnp2, lns2, sq_on_dve=True,
                     warm=warm2)

        for _pool in (warm2_ps, ppr_ps, xres, lns2, lnp2):
            _pool.release()

        # -------- Phase 5: FFN --------------------------------------------
        oup = tc.alloc_tile_pool(name="oup", bufs=3)
        pu_ps = tc.alloc_tile_pool(name="pu", bufs=2, space="PSUM")
        py_ps = tc.alloc_tile_pool(name="py", bufs=2, space="PSUM")

        for mb in range(MB):
            for qc2 in range(2):
                pu = pu_ps.tile([P, 512], f32, tag="pu", name=f"pu_{mb}_{qc2}")
                for eo in range(EO):
                    nc.tensor.matmul(
                        pu[:], w1_t[:, eo, mb * P:(mb + 1) * P],
                        h2T_g[qc2][:, eo, :],
                        start=(eo == 0), stop=(eo == EO - 1))
                nc.vector.tensor_scalar(
                    uT[:, mb, qc2 * 512:(qc2 + 1) * 512], pu[:],
                    b1c[:, mb:mb + 1], 0.0, ALU.add, ALU.max)
        for qb in range(NQ):
            x2r = oup.tile([P, C], f32, tag="x2r", name=f"x2r_{qb}")
            nc.sync.dma_start(x2r[:], x2_d.ap()[:, qb, :])
            for fo in range(2):
                py = py_ps.tile([P, 384], f32, tag="py", name=f"py_{qb}_{fo}")
                for mp in range(MB):
                    nc.tensor.matmul(
                        py[:], uT[:, mp, qb * P:(qb + 1) * P],
                        w2_t[:, mp, fo * 384:(fo + 1) * 384],
                        start=(mp == 0), stop=False)
                nc.tensor.matmul(py[:], ones1[:],
                                 b2r[:, fo * 384:(fo + 1) * 384],
                                 start=False, stop=True)
                ot = oup.tile([P, 384], f32, tag="ot", name=f"ot_{qb}_{fo}")
                nc.vector.tensor_tensor(
                    ot[:], py[:], x2r[:, fo * 384:(fo + 1) * 384], ALU.add)
                nc.sync.dma_start(
                    out_d.ap()[qb * P:(qb + 1) * P,
                               fo * 384:(fo + 1) * 384], ot[:])

        for _pool in (py_ps, pu_ps, oup, ffn, late, pers):
            _pool.release()

    return nc


def prepare_in_maps(inputs):
    """Build the 8 per-core input maps from the full problem inputs."""
    x = np.asarray(inputs["x"], np.float32)
    wq = np.asarray(inputs["wq"], np.float32)
    wk = np.asarray(inputs["wk"], np.float32)
    wv = np.asarray(inputs["wv"], np.float32)
    w_proj = np.asarray(inputs["w_proj"], np.float32)
    b_proj = np.asarray(inputs["b_proj"], np.float32)
    w1 = np.asarray(inputs["w1"], np.float32)
    b1 = np.asarray(inputs["b1"], np.float32)
    w2 = np.asarray(inputs["w2"], np.float32)
    b2 = np.asarray(inputs["b2"], np.float32)
    g1 = np.asarray(inputs["ln1_g"], np.float32)
    be1 = np.asarray(inputs["ln1_b"], np.float32)
    g2 = np.asarray(inputs["ln2_g"], np.float32)
    be2 = np.asarray(inputs["ln2_b"], np.float32)

    bf = ml_dtypes.bfloat16
    wq_r = wq.transpose(1, 0, 2).reshape(C, C)       # [c, h*d]
    wk_r = wk.transpose(1, 0, 2).reshape(C, C)
    wv_r = wv.transpose(1, 0, 2).reshape(C, C)
    wq_g = (g1[:, None] * wq_r).astype(bf)
    wk_g = (g1[:, None] * wk_r).astype(bf)
    wv_g = (g1[:, None] * wv_r).astype(bf)
    bq = (be1 @ wq_r).reshape(HP, P).T.copy().astype(np.float32)   # [128, hp]
    bk = (be1 @ wk_r).reshape(HP, P).T.copy().astype(np.float32)
    bv = (be1 @ wv_r).reshape(1, C).astype(bf)
    w1_g = (g2[:, None] * w1).astype(bf)
    b1f = (b1 + be2 @ w1).reshape(MB, P).T.copy().astype(np.float32)  # [128, mb]
    wp16 = w_proj.astype(bf)
    w2_16 = w2.astype(bf)
    bp = b_proj.reshape(1, C).astype(bf)
    b2r = b2.reshape(1, C).astype(bf)

    ci = np.arange(P)[:, None]
    qi = np.arange(P)[None, :]
    tri = (ci <= qi).astype(np.float32)          # visible where c <= q
    m_par = [
        np.stack([tri, np.zeros((P, P), np.float32)], 0),   # parity 0
        np.stack([np.ones((P, P), np.float32), tri], 0),    # parity 1
    ]

    eab = np.zeros((12, 24, P), np.float32)
    for i in range(12):
        hp_, qc_ = i // 2, i % 2
        eab[i, hp_ * 4 + qc_ * 2 + 0, 0:64] = 1.0
        eab[i, hp_ * 4 + qc_ * 2 + 1, 64:128] = 1.0
    eab16 = eab.astype(bf)

    in_maps = []
    for core in range(8):
        b, p = core // 2, core % 2
        xf = np.ascontiguousarray(x[b])
        xq = np.ascontiguousarray(
            x[b].reshape(NB, P, C)[p::2].reshape(TQ, C))
        mk = np.ascontiguousarray(m_par[p].transpose(1, 0, 2)).astype(bf)
        s0 = np.full((P, 1), 1.0 - p, np.float32)
        s1 = np.full((P, 1), float(p), np.float32)
        in_maps.append({
            "xq": xq, "xf": xf,
            "wq": wq_g, "wk": wk_g, "wv": wv_g, "wp": wp16,
            "w1": w1_g, "w2": w2_16,
            "bq": bq, "bk": bk, "bv": bv, "bp": bp, "b1": b1f, "b2": b2r,
            "mk": mk, "s0": s0, "s1": s1, "eab": eab16,
        })
    return in_maps


def assemble_output(results):
    """Reassemble the 8 per-core [1024, 768] outputs into [4, 2048, 768]."""
    out = np.empty((4, T, C), np.float32)
    for core in range(8):
        b, p = core // 2, core % 2
        blocks = results[core]["out"].reshape(NQ, P, C)
        ov = out[b].reshape(NB, P, C)
        ov[p::2] = blocks
    return out


_CACHED_NC = None


def kernel(**inputs) -> np.ndarray:
    global _CACHED_NC
    from concourse.bass_utils import run_bass_kernel_spmd

    if _CACHED_NC is None:
        nc = build_program()
        split_multiwait_instructions(nc)
        _CACHED_NC = nc
    in_maps = prepare_in_maps(inputs)
    res = run_bass_kernel_spmd(_CACHED_NC, in_maps, list(range(8)))
    return assemble_output(res.results)


# revision 22
# speedup vs baseline: 1.0296x; 1.0021x over previous
"""Trainium2 Bass kernel for a dense transformer block (attention + FFN).

Sharding: data-parallel over (batch, sequence-parity). 8 cores = 4 batches x 2
parity groups. Core c handles batch b = c//2 and the 128-row blocks of parity
p = c%2 (blocks p, p+2, ..., p+14) as query rows; K/V are computed for the
full sequence of the batch on-core (no collectives). The causal structure is
made SPMD-uniform by computing, for query block i, key blocks j <= 2i+1 and
masking with a small per-core multiplicative mask input.

On-chip layout: activations are kept feature-major (transposed) where matmuls
need them as stationary/moving operands; scores are computed transposed
(S^T[c,q]) so softmax probabilities feed the attention*V matmul directly with
no transpose, and the row-sum comes for free from an appended ones-column in
the V stationary operand.
"""
import sys

sys.path.insert(0, '/opt/trn_rl_repo')

import numpy as np
import ml_dtypes

import bass_rust
import concourse.bass as bass
import concourse.tile as tile
from concourse import mybir

P = 128
T = 2048
TQ = 1024
C = 768
H = 12
D = 64
FF = 3072
EO = C // P          # 6
MB = FF // P         # 24
NB = T // P          # 16
NQ = TQ // P         # 8
HP = H // 2          # 6

f32 = mybir.dt.float32
bf16 = mybir.dt.bfloat16
AF = mybir.ActivationFunctionType
ALU = mybir.AluOpType


def split_multiwait_instructions(nc):
    """The installed walrus build rejects any instruction carrying more than
    one sync wait; hoist extra waits onto NoOps inserted before it on the
    same (serial) engine."""
    n_fixed = 0
    for f in nc.m.functions:
        for bb in f.blocks:
            insts = bb.instructions
            new_insts = []
            dirty = False
            for inst in insts:
                si = inst.sync_info
                waits = list(si.on_wait) if si and si.on_wait else []
                if len(waits) > 1:
                    for j, w in enumerate(waits[:-1]):
                        nop = bass_rust.InstNoOp(
                            name=f"{inst.name}_sw{j}", ins=[], outs=[]
                        )
                        nop.engine = inst.engine
                        nop.sync_info = bass_rust.SyncInfo(
                            on_wait=[w], on_update=[]
                        )
                        new_insts.append(nop)
                    si.on_wait = waits[-1:]
                    dirty = True
                    n_fixed += 1
                new_insts.append(inst)
            if dirty:
                bb.instructions = new_insts
    return n_fixed


def build_program():
    """Build the single SPMD program (identical on all 8 cores)."""
    nc = bass.Bass("TRN2", target_bir_lowering=False, debug=False,
                   num_devices=8)

    xq_d = nc.declare_dram_parameter("xq", [TQ, C], f32, isOutput=False)
    xf_d = nc.declare_dram_parameter("xf", [T, C], f32, isOutput=False)
    wq_d = nc.declare_dram_parameter("wq", [C, C], bf16, isOutput=False)
    wk_d = nc.declare_dram_parameter("wk", [C, C], bf16, isOutput=False)
    wv_d = nc.declare_dram_parameter("wv", [C, C], bf16, isOutput=False)
    wp_d = nc.declare_dram_parameter("wp", [C, C], bf16, isOutput=False)
    w1_d = nc.declare_dram_parameter("w1", [C, FF], bf16, isOutput=False)
    w2_d = nc.declare_dram_parameter("w2", [FF, C], bf16, isOutput=False)
    bq_d = nc.declare_dram_parameter("bq", [P, HP], f32, isOutput=False)
    bk_d = nc.declare_dram_parameter("bk", [P, HP], f32, isOutput=False)
    bv_d = nc.declare_dram_parameter("bv", [1, C], bf16, isOutput=False)
    bp_d = nc.declare_dram_parameter("bp", [1, C], bf16, isOutput=False)
    b1_d = nc.declare_dram_parameter("b1", [P, MB], f32, isOutput=False)
    b2_d = nc.declare_dram_parameter("b2", [1, C], bf16, isOutput=False)
    mk_d = nc.declare_dram_parameter("mk", [P, 2, P], bf16, isOutput=False)
    s0_d = nc.declare_dram_parameter("s0", [P, 1], f32, isOutput=False)
    s1_d = nc.declare_dram_parameter("s1", [P, 1], f32, isOutput=False)
    e_d = nc.declare_dram_parameter("eab", [12, 24, P], bf16, isOutput=False)
    out_d = nc.declare_dram_parameter("out", [TQ, C], f32, isOutput=True)

    x2_d = nc.dram_tensor("x2scratch", [P, NQ, C], f32)

    with tile.TileContext(nc) as tc:
        # --- pool stack (released LIFO) -------------------------------
        pers = tc.alloc_tile_pool(name="pers", bufs=1)
        late = tc.alloc_tile_pool(name="late", bufs=1)   # outT, wp, h2T
        attnp = tc.alloc_tile_pool(name="attnp", bufs=1)  # KT, QT, V

        ones1 = pers.tile([1, P], bf16)
        nc.vector.memset(ones1[:], 1.0)
        ones64 = pers.tile([1, 64], bf16)
        nc.vector.memset(ones64[:], 1.0)
        bqc = pers.tile([P, HP], f32)
        nc.sync.dma_start(bqc[:], bq_d[:])
        bkc = pers.tile([P, HP], f32)
        nc.sync.dma_start(bkc[:], bk_d[:])
        bvr = pers.tile([1, C], bf16)
        nc.sync.dma_start(bvr[:], bv_d[:])
        bpr = pers.tile([1, C], bf16)
        nc.sync.dma_start(bpr[:], bp_d[:])
        b1c = pers.tile([P, MB], f32)
        nc.sync.dma_start(b1c[:], b1_d[:])
        b2r = pers.tile([1, C], bf16)
        nc.sync.dma_start(b2r[:], b2_d[:])
        masks = pers.tile([P, 2, P], bf16)
        nc.sync.dma_start(masks[:], mk_d[:])
        s0c = pers.tile([P, 1], f32)
        nc.sync.dma_start(s0c[:], s0_d[:])
        s1c = pers.tile([P, 1], f32)
        nc.sync.dma_start(s1c[:], s1_d[:])
        eabt = pers.tile([24, 12, P], bf16)
        nc.sync.dma_start(eabt[:], e_d.ap().rearrange("i k p -> k i p"))

        outT_g = [late.tile([P, HP, 512], bf16, tag=f"outT{g}",
                             name=f"outT{g}") for g in range(2)]
        wp_t = late.tile([P, EO, C], bf16)
        h2T_g = [late.tile([P, EO, 512], bf16, tag=f"h2T{g}", name=f"h2T{g}")
                 for g in range(2)]

        KT = attnp.tile([P, HP, T], bf16)
        QT = attnp.tile([P, HP, TQ], bf16)
        V = attnp.tile([P, NB, H, 65], bf16)
        nc.vector.memset(V[:], 1.0)

        def ln_block(x_ap, dst, col, nm, lnp, lns, sq_on_dve=False,
                     warm=None):
            # x_ap: [128, 768] (DRAM or SBUF) -> normalized bf16 rows,
            # transposed into dst[:, :, col:col+128]
            if x_ap.space == bass.MemorySpace.DRAM:
                x_t = lnp.tile([P, C], f32, tag="ln_x", name=f"lnx_{nm}")
                nc.sync.dma_start(x_t[:], x_ap)
            else:
                x_t = x_ap
            s1 = lns.tile([P, 1], f32, tag="ln_s1", name=f"s1_{nm}")
            nc.vector.tensor_reduce(s1[:], x_t[:], mybir.AxisListType.X,
                                    ALU.add)
            sq = lnp.tile([P, C], bf16, tag="ln_sq", name=f"sq_{nm}")
            s2 = lns.tile([P, 1], f32, tag="ln_s2", name=f"s2_{nm}")
            nc.scalar.activation(sq[:], x_t[:], AF.Square, accum_out=s2[:])
            mu = lns.tile([P, 1], f32, tag="ln_mu", name=f"mu_{nm}")
            nc.vector.tensor_scalar_mul(mu[:], s1[:], 1.0 / C)
            mu2 = lns.tile([P, 1], f32, tag="ln_mu2", name=f"mu2_{nm}")
            nc.vector.tensor_scalar(mu2[:], mu[:], mu[:], None, ALU.mult)
            ve = lns.tile([P, 1], f32, tag="ln_ve", name=f"ve_{nm}")
            nc.vector.tensor_scalar(ve[:], s2[:], 1.0 / C, 1e-5,
                                    ALU.mult, ALU.add)
            nc.vector.tensor_scalar(ve[:], ve[:], mu2[:], None, ALU.subtract)
            sd = lns.tile([P, 1], f32, tag="ln_sd", name=f"sd_{nm}")
            nc.scalar.activation(sd[:], ve[:], AF.Sqrt)
            rstd = lns.tile([P, 1], f32, tag="ln_rstd", name=f"rstd_{nm}")
            nc.vector.reciprocal(rstd[:], sd[:])
            nbias = lns.tile([P, 1], f32, tag="ln_nb", name=f"nb_{nm}")
            nc.vector.tensor_scalar(nbias[:], mu[:], rstd[:], -1.0,
                                    ALU.mult, ALU.mult)
            z = lnp.tile([P, C], bf16, tag="ln_z", name=f"z_{nm}")
            nc.scalar.activation(z[:], x_t[:], AF.Identity,
                                 bias=nbias[:], scale=rstd[:])
            nc.sync.dma_start_transpose(dst[:, :, col:col + P], z[:])
            if warm is not None:
                # cheap dependency-chained matmul to keep the PE clock warm
                nc.tensor.matmul(warm[:, 0:512], ones1[:], z[0:1, 0:512],
                                 start=True, stop=True)

        # ---------------- Phase 1+2: LN1 and QKV projections -------------
        w13 = tc.alloc_tile_pool(name="w13", bufs=1)
        lnp = tc.alloc_tile_pool(name="lnp", bufs=3)
        lns = tc.alloc_tile_pool(name="lns", bufs=4)
        pq_ps = tc.alloc_tile_pool(name="pq_ps", bufs=3, space="PSUM")
        pv_ps = tc.alloc_tile_pool(name="pv_ps", bufs=3, space="PSUM")
        warm_ps = tc.alloc_tile_pool(name="warm_ps", bufs=1, space="PSUM")

        wq_t = w13.tile([P, EO, C], bf16)
        nc.sync.dma_start(wq_t[:], wq_d.ap().rearrange("(o p) f -> p o f", p=P))
        wk_t = w13.tile([P, EO, C], bf16)
        nc.sync.dma_start(wk_t[:], wk_d.ap().rearrange("(o p) f -> p o f", p=P))
        wv_t = w13.tile([P, EO, C], bf16)
        nc.sync.dma_start(wv_t[:], wv_d.ap().rearrange("(o p) f -> p o f", p=P))
        hT_g = [w13.tile([P, EO, 512], bf16, tag=f"hT{g}", name=f"hT{g}")
                for g in range(4)]
        hqT_g = [w13.tile([P, EO, 512], bf16, tag=f"hqT{g}", name=f"hqT{g}")
                 for g in range(2)]

        warm1 = warm_ps.tile([P, 512], f32, tag="warm", name="warm1")
        for b in range(NB):
            ln_block(xf_d.ap()[b * P:(b + 1) * P, :], hT_g[b // 4],
                     (b % 4) * P, f"f{b}", lnp, lns, warm=warm1)
        # hqT = parity-selected columns of hT (s0/s1 are 1/0 per core parity)
        hq_tmp = w13.tile([P, EO, 512], bf16)
        for g in range(2):
            for half in range(2):
                blk = hT_g[2 * g + half][:].rearrange(
                    "p o (b two t) -> p o b two t", two=2, t=P)
                dstv = hqT_g[g][:, :, half * 256:(half + 1) * 256].rearrange(
                    "p o (b t) -> p o b t", t=P)
                tmpv = hq_tmp[:, :, half * 256:(half + 1) * 256].rearrange(
                    "p o (b t) -> p o b t", t=P)
                nc.vector.tensor_scalar(tmpv, blk[:, :, :, 0, :], s0c[:],
                                        None, ALU.mult)
                nc.vector.tensor_scalar(dstv, blk[:, :, :, 1, :], s1c[:],
                                        None, ALU.mult)
            nc.vector.tensor_tensor(hqT_g[g][:], hqT_g[g][:],
                                    hq_tmp[:], ALU.add)

        # Q^T (own rows) and K^T (full rows), 2 heads col-packed per tile
        for hp in range(HP):
            for qc in range(2):
                pq = pq_ps.tile([P, 512], f32, tag="pqkt", name=f"pq_{hp}_{qc}")
                for eo in range(EO):
                    for ab in range(2):
                        nc.tensor.matmul(
                            pq[64 * ab:64 * (ab + 1), :],
                            wq_t[:, eo, hp * P + 64 * ab:hp * P + 64 * (ab + 1)],
                            hqT_g[qc][:, eo, :],
                            start=(eo == 0), stop=(eo == EO - 1),
                            tile_position=(0, 64 * ab),
                            skip_group_check=True)
                nc.scalar.activation(QT[:, hp, qc * 512:(qc + 1) * 512],
                                     pq[:], AF.Identity,
                                     bias=bqc[:, hp:hp + 1])
            for cc in range(4):
                pk = pq_ps.tile([P, 512], f32, tag="pqkt", name=f"pk_{hp}_{cc}")
                for eo in range(EO):
                    for ab in range(2):
                        nc.tensor.matmul(
                            pk[64 * ab:64 * (ab + 1), :],
                            wk_t[:, eo, hp * P + 64 * ab:hp * P + 64 * (ab + 1)],
                            hT_g[cc][:, eo, :],
                            start=(eo == 0), stop=(eo == EO - 1),
                            tile_position=(0, 64 * ab),
                            skip_group_check=True)
                nc.scalar.activation(KT[:, hp, cc * 512:(cc + 1) * 512],
                                     pk[:], AF.Identity,
                                     bias=bkc[:, hp:hp + 1])

        # V natural [c, f] with a ones column at f=64 per head
        for cb in range(NB):
            for fo in range(2):
                pv = pv_ps.tile([P, 384], f32, tag="pv", name=f"pv_{cb}_{fo}")
                for eo in range(EO):
                    nc.tensor.matmul(
                        pv[:], hT_g[cb // 4][:, eo, (cb % 4) * P:(cb % 4 + 1) * P],
                        wv_t[:, eo, fo * 384:(fo + 1) * 384],
                        start=(eo == 0), stop=False)
                nc.tensor.matmul(pv[:], ones1[:],
                                 bvr[:, fo * 384:(fo + 1) * 384],
                                 start=False, stop=True)
                nc.scalar.activation(
                    V[:, cb, fo * 6:(fo + 1) * 6, 0:64],
                    pv[:].rearrange("p (g d) -> p g d", g=6),
                    AF.Identity)

        for _pool in (warm_ps, pv_ps, pq_ps, lns, lnp, w13):
            _pool.release()

        # prefetch the projection weight while attention runs
        nc.sync.dma_start(wp_t[:], wp_d.ap().rearrange("(o p) f -> p o f", p=P))

        # ---------------- Phase 3: attention -----------------------------
        pr = tc.alloc_tile_pool(name="pr", bufs=3)
        rsn = tc.alloc_tile_pool(name="rsn", bufs=2)
        psc_ps = tc.alloc_tile_pool(name="psc", bufs=2, space="PSUM")
        pso_ps = tc.alloc_tile_pool(name="pso", bufs=2, space="PSUM")

        # rs16_pad rows 0:24 hold bf16 row-sums; the rest are 1.0 filler so
        # the padded reciprocal stays finite.
        rs16_pad = rsn.tile([P, 512], bf16, name="rs16_pad")
        nc.vector.memset(rs16_pad[:], 1.0)

        for hp in range(HP):
            for qc in range(2):
                q0 = qc * 512
                poA = pso_ps.tile([P, 512], f32, tag="poA", name=f"poA_{hp}_{qc}")
                poB = pso_ps.tile([P, 512], f32, tag="poB", name=f"poB_{hp}_{qc}")
                po_t = [poA, poB]
                jmax = 8 if qc == 0 else 16
                for j in range(jmax):
                    qsj = (j // 2) * P
                    qs = max(qsj, q0)
                    off = qs - q0
                    N = 512 - off
                    psc = psc_ps.tile([P, 2, 512], f32, tag="psc",
                                      name=f"psc_{hp}_{qc}_{j}")
                    for ab in range(2):
                        nc.tensor.matmul(
                            psc[:, ab, off:off + N],
                            KT[64 * ab:64 * (ab + 1), hp, j * P:(j + 1) * P],
                            QT[64 * ab:64 * (ab + 1), hp, qs:qs + N],
                            start=True, stop=True,
                            tile_position=(64 * ab, 0))
                    probs = pr.tile([P, 2, 512], bf16, tag="probs",
                                    name=f"pb_{hp}_{qc}_{j}")
                    nc.scalar.activation(probs[:, :, off:off + N],
                                         psc[:, :, off:off + N],
                                         AF.Exp, scale=0.125)
                    if qs == qsj:
                        nc.vector.tensor_tensor(
                            probs[:, :, off:off + P],
                            probs[:, :, off:off + P],
                            masks[:, j % 2, None, :].to_broadcast(
                                (P, 2, P)), ALU.mult)
                    for ab in range(2):
                        nc.tensor.matmul(
                            po_t[ab][0:65, off:off + N],
                            V[:, j, 2 * hp + ab, :],
                            probs[:, ab, off:off + N],
                            start=(j == 0), stop=(j == jmax - 1))
                # evict unnormalized out^T and stash the row-sums (row 64)
                for ab in range(2):
                    nc.vector.tensor_copy(
                        out=outT_g[qc][64 * ab:64 * (ab + 1), hp, :],
                        in_=po_t[ab][0:64, :])
                    rstmp = rsn.tile([1, 512], bf16, tag="rstmp",
                                     name=f"rst_{hp}_{qc}_{ab}")
                    nc.vector.tensor_copy(out=rstmp[:],
                                          in_=po_t[ab][64:65, :])
                    k = hp * 4 + qc * 2 + ab
                    nc.sync.dma_start(rs16_pad[k:k + 1, :], rstmp[:])

        for _pool in (pso_ps, psc_ps):
            _pool.release()

        # ---- deferred normalization of outT ------------------------------
        nrm_ps = tc.alloc_tile_pool(name="nrm", bufs=2, space="PSUM")
        rsT = rsn.tile([P, 4, P], bf16, name="rsT")
        for c in range(4):
            nc.scalar.dma_start_transpose(rsT[:, c, :],
                                          rs16_pad[:, c * P:(c + 1) * P])
        rsTf = rsn.tile([P, 4 * P], f32, name="rsTf")
        nc.vector.tensor_copy(out=rsTf[:], in_=rsT[:])
        recTf = rsn.tile([P, 4 * P], f32, name="recTf")
        nc.vector.reciprocal(recTf[:], rsTf[:])
        recT16 = rsn.tile([P, 4, P], bf16, name="recT16")
        nc.vector.tensor_copy(out=recT16[:], in_=recTf[:])
        rec16_pad = rsn.tile([P, 512], bf16, name="rec16_pad")
        for c in range(4):
            nc.scalar.dma_start_transpose(rec16_pad[:, c * P:(c + 1) * P],
                                          recT16[:, c, :])
        for qc in range(2):
            for hp in range(HP):
                pb = nrm_ps.tile([P, 512], f32, tag="pbn",
                                 name=f"pbn_{hp}_{qc}")
                nc.tensor.matmul(pb[:], eabt[:, hp * 2 + qc, :],
                                 rec16_pad[0:24, :],
                                 start=True, stop=True)
                nc.vector.tensor_tensor(
                    outT_g[qc][:, hp, :], outT_g[qc][:, hp, :],
                    pb[:], ALU.mult)

        for _pool in (nrm_ps, rsn, pr):
            _pool.release()
        attnp.release()

        # -------- Phase 4: projection + residual + LN2 --------------------
        # FFN weights prefetch during proj (pool allocated below the proj
        # transients so it survives into the FFN phase)
        ffn = tc.alloc_tile_pool(name="ffn", bufs=1)
        w1_t = ffn.tile([P, EO, FF], bf16)
        for eo in range(EO):
            nc.gpsimd.dma_start(
                w1_t[:, eo, :], w1_d.ap()[eo * P:(eo + 1) * P, :])
        w2_t = ffn.tile([P, MB, C], bf16)
        for mp in range(MB):
            nc.gpsimd.dma_start(
                w2_t[:, mp, :], w2_d.ap()[mp * P:(mp + 1) * P, :])
        uT_g = [ffn.tile([P, MB, 512], bf16, tag=f"uT{g}", name=f"uT{g}")
                for g in range(2)]

        lnp2 = tc.alloc_tile_pool(name="lnp2", bufs=3)
        lns2 = tc.alloc_tile_pool(name="lns2", bufs=4)
        xres = tc.alloc_tile_pool(name="xres", bufs=3)
        ppr_ps = tc.alloc_tile_pool(name="ppr", bufs=2, space="PSUM")
        warm2_ps = tc.alloc_tile_pool(name="warm2_ps", bufs=1, space="PSUM")

        warm2 = warm2_ps.tile([P, 512], f32, tag="warm2", name="warm2")
        for qb in range(NQ):
            xqb = xres.tile([P, C], f32, tag="xqb", name=f"xqb_{qb}")
            nc.sync.dma_start(xqb[:], xq_d.ap()[qb * P:(qb + 1) * P, :])
            x2b = xres.tile([P, C], f32, tag="x2b", name=f"x2b_{qb}")
            for fo in range(2):
                pp = ppr_ps.tile([P, 384], f32, tag="ppr", name=f"pp_{qb}_{fo}")
                for fp in range(EO):
                    nc.tensor.matmul(
                        pp[:],
                        outT_g[qb // 4][:, fp, (qb % 4) * P:(qb % 4 + 1) * P],
                        wp_t[:, fp, fo * 384:(fo + 1) * 384],
                        start=(fp == 0), stop=False)
                nc.tensor.matmul(pp[:], ones1[:],
                                 bpr[:, fo * 384:(fo + 1) * 384],
                                 start=False, stop=True)
                nc.vector.tensor_tensor(
                    x2b[:, fo * 384:(fo + 1) * 384], pp[:],
                    xqb[:, fo * 384:(fo + 1) * 384], ALU.add)
            nc.sync.dma_start(x2_d.ap()[:, qb, :], x2b[:])
            ln_block(x2b[:], h2T_g[qb // 4], (qb % 4) * P,
                     f"x2{qb}", lnp2, lns2, sq_on_dve=True,
                     warm=warm2)

        for _pool in (warm2_ps, ppr_ps, xres, lns2, lnp2):
            _pool.release()

        # -------- Phase 5: FFN --------------------------------------------
        oup = tc.alloc_tile_pool(name="oup", bufs=3)
        pu_ps = tc.alloc_tile_pool(name="pu", bufs=2, space="PSUM")
        py_ps = tc.alloc_tile_pool(name="py", bufs=2, space="PSUM")

        for qc2 in range(2):
            for mb in range(MB):
                pu = pu_ps.tile([P, 512], f32, tag="pu", name=f"pu_{mb}_{qc2}")
                for eo in range(EO):
                    nc.tensor.matmul(
                        pu[:], w1_t[:, eo, mb * P:(mb + 1) * P],
                        h2T_g[qc2][:, eo, :],
                        start=(eo == 0), stop=(eo == EO - 1))
                nc.vector.tensor_scalar(
                    uT_g[qc2][:, mb, :], pu[:],
                    b1c[:, mb:mb + 1], 0.0, ALU.add, ALU.max)
        for qb in range(NQ):
            x2r = oup.tile([P, C], f32, tag="x2r", name=f"x2r_{qb}")
            nc.sync.dma_start(x2r[:], x2_d.ap()[:, qb, :])
            for fo in range(2):
                py = py_ps.tile([P, 384], f32, tag="py", name=f"py_{qb}_{fo}")
                for mp in range(MB):
                    nc.tensor.matmul(
                        py[:],
                        uT_g[qb // 4][:, mp, (qb % 4) * P:(qb % 4 + 1) * P],
                        w2_t[:, mp, fo * 384:(fo + 1) * 384],
                        start=(mp == 0), stop=False)
                nc.tensor.matmul(py[:], ones1[:],
                                 b2r[:, fo * 384:(fo + 1) * 384],
                                 start=False, stop=True)
                ot = oup.tile([P, 384], f32, tag="ot", name=f"ot_{qb}_{fo}")
                nc.vector.tensor_tensor(
                    ot[:], py[:], x2r[:, fo * 384:(fo + 1) * 384], ALU.add)
                nc.sync.dma_start(
                    out_d.ap()[qb * P:(qb + 1) * P,
                               fo * 384:(fo + 1) * 384], ot[:])

        for _pool in (py_ps, pu_ps, oup, ffn, late, pers):
            _pool.release()

    return nc


def prepare_in_maps(inputs):
    """Build the 8 per-core input maps from the full problem inputs."""
    x = np.asarray(inputs["x"], np.float32)
    wq = np.asarray(inputs["wq"], np.float32)
    wk = np.asarray(inputs["wk"], np.float32)
    wv = np.asarray(inputs["wv"], np.float32)
    w_proj = np.asarray(inputs["w_proj"], np.float32)
    b_proj = np.asarray(inputs["b_proj"], np.float32)
    w1 = np.asarray(inputs["w1"], np.float32)
    b1 = np.asarray(inputs["b1"], np.float32)
    w2 = np.asarray(inputs["w2"], np.float32)
    b2 = np.asarray(inputs["b2"], np.float32)
    g1 = np.asarray(inputs["ln1_g"], np.float32)
    be1 = np.asarray(inputs["ln1_b"], np.float32)
    g2 = np.asarray(inputs["ln2_g"], np.float32)
    be2 = np.asarray(inputs["ln2_b"], np.float32)

    bf = ml_dtypes.bfloat16
    wq_r = wq.transpose(1, 0, 2).reshape(C, C)       # [c, h*d]
    wk_r = wk.transpose(1, 0, 2).reshape(C, C)
    wv_r = wv.transpose(1, 0, 2).reshape(C, C)
    wq_g = (g1[:, None] * wq_r).astype(bf)
    wk_g = (g1[:, None] * wk_r).astype(bf)
    wv_g = (g1[:, None] * wv_r).astype(bf)
    bq = (be1 @ wq_r).reshape(HP, P).T.copy().astype(np.float32)   # [128, hp]
    bk = (be1 @ wk_r).reshape(HP, P).T.copy().astype(np.float32)
    bv = (be1 @ wv_r).reshape(1, C).astype(bf)
    w1_g = (g2[:, None] * w1).astype(bf)
    b1f = (b1 + be2 @ w1).reshape(MB, P).T.copy().astype(np.float32)  # [128, mb]
    wp16 = w_proj.astype(bf)
    w2_16 = w2.astype(bf)
    bp = b_proj.reshape(1, C).astype(bf)
    b2r = b2.reshape(1, C).astype(bf)

    ci = np.arange(P)[:, None]
    qi = np.arange(P)[None, :]
    tri = (ci <= qi).astype(np.float32)          # visible where c <= q
    m_par = [
        np.stack([tri, np.zeros((P, P), np.float32)], 0),   # parity 0
        np.stack([np.ones((P, P), np.float32), tri], 0),    # parity 1
    ]

    eab = np.zeros((12, 24, P), np.float32)
    for i in range(12):
        hp_, qc_ = i // 2, i % 2
        eab[i, hp_ * 4 + qc_ * 2 + 0, 0:64] = 1.0
        eab[i, hp_ * 4 + qc_ * 2 + 1, 64:128] = 1.0
    eab16 = eab.astype(bf)

    in_maps = []
    for core in range(8):
        b, p = core // 2, core % 2
        xf = np.ascontiguousarray(x[b])
        xq = np.ascontiguousarray(
            x[b].reshape(NB, P, C)[p::2].reshape(TQ, C))
        mk = np.ascontiguousarray(m_par[p].transpose(1, 0, 2)).astype(bf)
        s0 = np.full((P, 1), 1.0 - p, np.float32)
        s1 = np.full((P, 1), float(p), np.float32)
        in_maps.append({
            "xq": xq, "xf": xf,
            "wq": wq_g, "wk": wk_g, "wv": wv_g, "wp": wp16,
            "w1": w1_g, "w2": w2_16,
            "bq": bq, "bk": bk, "bv": bv, "bp": bp, "b1": b1f, "b2": b2r,
            "mk": mk, "s0": s0, "s1": s1, "eab": eab16,
        })
    return in_maps


def assemble_output(results):
    """Reassemble the 8 per-core [1024, 768] outputs into [4, 2048, 768]."""
    out = np.empty((4, T, C), np.float32)
    for core in range(8):
        b, p = core // 2, core % 2
        blocks = results[core]["out"].reshape(NQ, P, C)
        ov = out[b].reshape(NB, P, C)
        ov[p::2] = blocks
    return out


_CACHED_NC = None


def kernel(**inputs) -> np.ndarray:
    global _CACHED_NC
    from concourse.bass_utils import run_bass_kernel_spmd

    if _CACHED_NC is None:
        nc = build_program()
        split_multiwait_instructions(nc)
        _CACHED_NC = nc
    in_maps = prepare_in_maps(inputs)
    res = run_bass_kernel_spmd(_CACHED_NC, in_maps, list(range(8)))
    return assemble_output(res.results)


# revision 23
# speedup vs baseline: 1.0568x; 1.0265x over previous
"""Trainium2 Bass kernel for a dense transformer block (attention + FFN).

Sharding: data-parallel over (batch, sequence-parity). 8 cores = 4 batches x 2
parity groups. Core c handles batch b = c//2 and the 128-row blocks of parity
p = c%2 (blocks p, p+2, ..., p+14) as query rows; K/V are computed for the
full sequence of the batch on-core (no collectives). The causal structure is
made SPMD-uniform by computing, for query block i, key blocks j <= 2i+1 and
masking with a small per-core multiplicative mask input.

On-chip layout: activations are kept feature-major (transposed) where matmuls
need them as stationary/moving operands; scores are computed transposed
(S^T[c,q]) so softmax probabilities feed the attention*V matmul directly with
no transpose, and the row-sum comes for free from an appended ones-column in
the V stationary operand.
"""
import sys

sys.path.insert(0, '/opt/trn_rl_repo')

import numpy as np
import ml_dtypes

import bass_rust
import concourse.bass as bass
import concourse.tile as tile
from concourse import mybir

P = 128
T = 2048
TQ = 1024
C = 768
H = 12
D = 64
FF = 3072
EO = C // P          # 6
MB = FF // P         # 24
NB = T // P          # 16
NQ = TQ // P         # 8
HP = H // 2          # 6

f32 = mybir.dt.float32
bf16 = mybir.dt.bfloat16
AF = mybir.ActivationFunctionType
ALU = mybir.AluOpType


def split_multiwait_instructions(nc):
    """The installed walrus build rejects any instruction carrying more than
    one sync wait; hoist extra waits onto NoOps inserted before it on the
    same (serial) engine."""
    n_fixed = 0
    for f in nc.m.functions:
        for bb in f.blocks:
            insts = bb.instructions
            new_insts = []
            dirty = False
            for inst in insts:
                si = inst.sync_info
                waits = list(si.on_wait) if si and si.on_wait else []
                if len(waits) > 1:
                    for j, w in enumerate(waits[:-1]):
                        nop = bass_rust.InstNoOp(
                            name=f"{inst.name}_sw{j}", ins=[], outs=[]
                        )
                        nop.engine = inst.engine
                        nop.sync_info = bass_rust.SyncInfo(
                            on_wait=[w], on_update=[]
                        )
                        new_insts.append(nop)
                    si.on_wait = waits[-1:]
                    dirty = True
                    n_fixed += 1
                new_insts.append(inst)
            if dirty:
                bb.instructions = new_insts
    return n_fixed


def build_program():
    """Build the single SPMD program (identical on all 8 cores)."""
    nc = bass.Bass("TRN2", target_bir_lowering=False, debug=False,
                   num_devices=8)

    xq_d = nc.declare_dram_parameter("xq", [TQ, C], f32, isOutput=False)
    xf_d = nc.declare_dram_parameter("xf", [T, C], f32, isOutput=False)
    wq_d = nc.declare_dram_parameter("wq", [C, C], bf16, isOutput=False)
    wk_d = nc.declare_dram_parameter("wk", [C, C], bf16, isOutput=False)
    wv_d = nc.declare_dram_parameter("wv", [C, C], bf16, isOutput=False)
    wp_d = nc.declare_dram_parameter("wp", [C, C], bf16, isOutput=False)
    w1_d = nc.declare_dram_parameter("w1", [C, FF], bf16, isOutput=False)
    w2_d = nc.declare_dram_parameter("w2", [FF, C], bf16, isOutput=False)
    bq_d = nc.declare_dram_parameter("bq", [P, HP], f32, isOutput=False)
    bk_d = nc.declare_dram_parameter("bk", [P, HP], f32, isOutput=False)
    bv_d = nc.declare_dram_parameter("bv", [1, C], bf16, isOutput=False)
    bp_d = nc.declare_dram_parameter("bp", [1, C], bf16, isOutput=False)
    b1_d = nc.declare_dram_parameter("b1", [P, MB], f32, isOutput=False)
    b2_d = nc.declare_dram_parameter("b2", [1, C], bf16, isOutput=False)
    mk_d = nc.declare_dram_parameter("mk", [P, 2, P], bf16, isOutput=False)
    s0_d = nc.declare_dram_parameter("s0", [P, 1], f32, isOutput=False)
    s1_d = nc.declare_dram_parameter("s1", [P, 1], f32, isOutput=False)
    e_d = nc.declare_dram_parameter("eab", [12, 24, P], bf16, isOutput=False)
    out_d = nc.declare_dram_parameter("out", [TQ, C], f32, isOutput=True)

    x2_d = nc.dram_tensor("x2scratch", [P, NQ, C], f32)

    with tile.TileContext(nc) as tc:
        # --- pool stack (released LIFO) -------------------------------
        pers = tc.alloc_tile_pool(name="pers", bufs=1)
        late = tc.alloc_tile_pool(name="late", bufs=1)   # outT, wp, h2T
        attnp = tc.alloc_tile_pool(name="attnp", bufs=1)  # KT, QT, V

        ones1 = pers.tile([1, P], bf16)
        nc.vector.memset(ones1[:], 1.0)
        ones64 = pers.tile([1, 64], bf16)
        nc.vector.memset(ones64[:], 1.0)
        bqc = pers.tile([P, HP], f32)
        nc.sync.dma_start(bqc[:], bq_d[:])
        bkc = pers.tile([P, HP], f32)
        nc.sync.dma_start(bkc[:], bk_d[:])
        bvr = pers.tile([1, C], bf16)
        nc.sync.dma_start(bvr[:], bv_d[:])
        bpr = pers.tile([1, C], bf16)
        nc.sync.dma_start(bpr[:], bp_d[:])
        b1c = pers.tile([P, MB], f32)
        nc.sync.dma_start(b1c[:], b1_d[:])
        b2r = pers.tile([1, C], bf16)
        nc.sync.dma_start(b2r[:], b2_d[:])
        masks = pers.tile([P, 2, P], bf16)
        nc.sync.dma_start(masks[:], mk_d[:])
        s0c = pers.tile([P, 1], f32)
        nc.sync.dma_start(s0c[:], s0_d[:])
        s1c = pers.tile([P, 1], f32)
        nc.sync.dma_start(s1c[:], s1_d[:])
        eabt = pers.tile([24, 12, P], bf16)
        nc.sync.dma_start(eabt[:], e_d.ap().rearrange("i k p -> k i p"))

        outT_g = [late.tile([P, HP, 512], bf16, tag=f"outT{g}",
                             name=f"outT{g}") for g in range(2)]
        wp_t = late.tile([P, EO, C], bf16)
        h2T_g = [late.tile([P, EO, 512], bf16, tag=f"h2T{g}", name=f"h2T{g}")
                 for g in range(2)]

        KT_h = [attnp.tile([P, T], bf16, tag=f"KT{h}", name=f"KT{h}")
                for h in range(HP)]
        QT_h = [attnp.tile([P, TQ], bf16, tag=f"QT{h}", name=f"QT{h}")
                for h in range(HP)]
        V = attnp.tile([P, NB, H, 65], bf16)
        nc.vector.memset(V[:], 1.0)

        def ln_block(x_ap, dst, col, nm, lnp, lns, sq_on_dve=False,
                     warm=None):
            # x_ap: [128, 768] (DRAM or SBUF) -> normalized bf16 rows,
            # transposed into dst[:, :, col:col+128]
            if x_ap.space == bass.MemorySpace.DRAM:
                x_t = lnp.tile([P, C], f32, tag="ln_x", name=f"lnx_{nm}")
                nc.sync.dma_start(x_t[:], x_ap)
            else:
                x_t = x_ap
            s1 = lns.tile([P, 1], f32, tag="ln_s1", name=f"s1_{nm}")
            nc.vector.tensor_reduce(s1[:], x_t[:], mybir.AxisListType.X,
                                    ALU.add)
            sq = lnp.tile([P, C], bf16, tag="ln_sq", name=f"sq_{nm}")
            s2 = lns.tile([P, 1], f32, tag="ln_s2", name=f"s2_{nm}")
            nc.scalar.activation(sq[:], x_t[:], AF.Square, accum_out=s2[:])
            mu = lns.tile([P, 1], f32, tag="ln_mu", name=f"mu_{nm}")
            nc.vector.tensor_scalar_mul(mu[:], s1[:], 1.0 / C)
            mu2 = lns.tile([P, 1], f32, tag="ln_mu2", name=f"mu2_{nm}")
            nc.vector.tensor_scalar(mu2[:], mu[:], mu[:], None, ALU.mult)
            ve = lns.tile([P, 1], f32, tag="ln_ve", name=f"ve_{nm}")
            nc.vector.tensor_scalar(ve[:], s2[:], 1.0 / C, 1e-5,
                                    ALU.mult, ALU.add)
            nc.vector.tensor_scalar(ve[:], ve[:], mu2[:], None, ALU.subtract)
            sd = lns.tile([P, 1], f32, tag="ln_sd", name=f"sd_{nm}")
            nc.scalar.activation(sd[:], ve[:], AF.Sqrt)
            rstd = lns.tile([P, 1], f32, tag="ln_rstd", name=f"rstd_{nm}")
            nc.vector.reciprocal(rstd[:], sd[:])
            nbias = lns.tile([P, 1], f32, tag="ln_nb", name=f"nb_{nm}")
            nc.vector.tensor_scalar(nbias[:], mu[:], rstd[:], -1.0,
                                    ALU.mult, ALU.mult)
            z = lnp.tile([P, C], bf16, tag="ln_z", name=f"z_{nm}")
            nc.scalar.activation(z[:], x_t[:], AF.Identity,
                                 bias=nbias[:], scale=rstd[:])
            nc.sync.dma_start_transpose(dst[:, :, col:col + P], z[:])
            if warm is not None:
                # cheap dependency-chained matmul to keep the PE clock warm
                nc.tensor.matmul(warm[:, 0:512], ones1[:], z[0:1, 0:512],
                                 start=True, stop=True)

        # ---------------- Phase 1+2: LN1 and QKV projections -------------
        w13 = tc.alloc_tile_pool(name="w13", bufs=1)
        pq_ps = tc.alloc_tile_pool(name="pq_ps", bufs=2, space="PSUM")
        lnp = tc.alloc_tile_pool(name="lnp", bufs=3)
        lns = tc.alloc_tile_pool(name="lns", bufs=4)
        pv_ps = tc.alloc_tile_pool(name="pv_ps", bufs=3, space="PSUM")
        warm_ps = tc.alloc_tile_pool(name="warm_ps", bufs=1, space="PSUM")

        wq_t = w13.tile([P, EO, C], bf16)
        nc.sync.dma_start(wq_t[:], wq_d.ap().rearrange("(o p) f -> p o f", p=P))
        wk_t = w13.tile([P, EO, C], bf16)
        nc.sync.dma_start(wk_t[:], wk_d.ap().rearrange("(o p) f -> p o f", p=P))
        wv_t = w13.tile([P, EO, C], bf16)
        nc.sync.dma_start(wv_t[:], wv_d.ap().rearrange("(o p) f -> p o f", p=P))
        hT_g = [w13.tile([P, EO, 512], bf16, tag=f"hT{g}", name=f"hT{g}")
                for g in range(4)]
        hqT_g = [w13.tile([P, EO, 512], bf16, tag=f"hqT{g}", name=f"hqT{g}")
                 for g in range(2)]

        warm1 = warm_ps.tile([P, 512], f32, tag="warm", name="warm1")
        for b in range(NB):
            ln_block(xf_d.ap()[b * P:(b + 1) * P, :], hT_g[b // 4],
                     (b % 4) * P, f"f{b}", lnp, lns, warm=warm1)
        # hqT = parity-selected columns of hT (s0/s1 are 1/0 per core parity)
        hq_tmp = w13.tile([P, EO, 512], bf16)
        for g in range(2):
            for half in range(2):
                blk = hT_g[2 * g + half][:].rearrange(
                    "p o (b two t) -> p o b two t", two=2, t=P)
                dstv = hqT_g[g][:, :, half * 256:(half + 1) * 256].rearrange(
                    "p o (b t) -> p o b t", t=P)
                tmpv = hq_tmp[:, :, half * 256:(half + 1) * 256].rearrange(
                    "p o (b t) -> p o b t", t=P)
                nc.vector.tensor_scalar(tmpv, blk[:, :, :, 0, :], s0c[:],
                                        None, ALU.mult)
                nc.vector.tensor_scalar(dstv, blk[:, :, :, 1, :], s1c[:],
                                        None, ALU.mult)
            nc.vector.tensor_tensor(hqT_g[g][:], hqT_g[g][:],
                                    hq_tmp[:], ALU.add)

        # V natural [c, f] with a ones column at f=64 per head
        for cb in range(NB):
            for fo in range(2):
                pv = pv_ps.tile([P, 384], f32, tag="pv", name=f"pv_{cb}_{fo}")
                for eo in range(EO):
                    nc.tensor.matmul(
                        pv[:], hT_g[cb // 4][:, eo, (cb % 4) * P:(cb % 4 + 1) * P],
                        wv_t[:, eo, fo * 384:(fo + 1) * 384],
                        start=(eo == 0), stop=False)
                nc.tensor.matmul(pv[:], ones1[:],
                                 bvr[:, fo * 384:(fo + 1) * 384],
                                 start=False, stop=True)
                nc.scalar.activation(
                    V[:, cb, fo * 6:(fo + 1) * 6, 0:64],
                    pv[:].rearrange("p (g d) -> p g d", g=6),
                    AF.Identity)

        for _pool in (warm_ps, pv_ps, lns, lnp):
            _pool.release()

        # prefetch the projection weight while attention runs
        nc.sync.dma_start(wp_t[:], wp_d.ap().rearrange("(o p) f -> p o f", p=P))

        # ---------------- Phase 3: attention -----------------------------
        pr = tc.alloc_tile_pool(name="pr", bufs=3)
        rsn = tc.alloc_tile_pool(name="rsn", bufs=2)
        psc_ps = tc.alloc_tile_pool(name="psc", bufs=2, space="PSUM")
        pso_ps = tc.alloc_tile_pool(name="pso", bufs=1, space="PSUM")

        # rs16_pad rows 0:24 hold bf16 row-sums; the rest are 1.0 filler so
        # the padded reciprocal stays finite.
        rs16_pad = rsn.tile([P, 512], bf16, name="rs16_pad")
        nc.vector.memset(rs16_pad[:], 1.0)

        for hp in range(HP):
            for qc in range(2):
                pq = pq_ps.tile([P, 512], f32, tag="pqkt", name=f"pq_{hp}_{qc}")
                for eo in range(EO):
                    for ab in range(2):
                        nc.tensor.matmul(
                            pq[64 * ab:64 * (ab + 1), :],
                            wq_t[:, eo, hp * P + 64 * ab:hp * P + 64 * (ab + 1)],
                            hqT_g[qc][:, eo, :],
                            start=(eo == 0), stop=(eo == EO - 1),
                            tile_position=(0, 64 * ab),
                            skip_group_check=True)
                nc.scalar.activation(QT_h[hp][:, qc * 512:(qc + 1) * 512],
                                     pq[:], AF.Identity,
                                     bias=bqc[:, hp:hp + 1])
            for cc in range(4):
                pk = pq_ps.tile([P, 512], f32, tag="pqkt", name=f"pk_{hp}_{cc}")
                for eo in range(EO):
                    for ab in range(2):
                        nc.tensor.matmul(
                            pk[64 * ab:64 * (ab + 1), :],
                            wk_t[:, eo, hp * P + 64 * ab:hp * P + 64 * (ab + 1)],
                            hT_g[cc][:, eo, :],
                            start=(eo == 0), stop=(eo == EO - 1),
                            tile_position=(0, 64 * ab),
                            skip_group_check=True)
                nc.scalar.activation(KT_h[hp][:, cc * 512:(cc + 1) * 512],
                                     pk[:], AF.Identity,
                                     bias=bkc[:, hp:hp + 1])
            for qc in range(2):
                q0 = qc * 512
                poA = pso_ps.tile([P, 512], f32, tag="poA", name=f"poA_{hp}_{qc}")
                poB = pso_ps.tile([P, 512], f32, tag="poB", name=f"poB_{hp}_{qc}")
                po_t = [poA, poB]
                jmax = 8 if qc == 0 else 16
                for j in range(jmax):
                    qsj = (j // 2) * P
                    qs = max(qsj, q0)
                    off = qs - q0
                    N = 512 - off
                    psc = psc_ps.tile([P, 2, 512], f32, tag="psc",
                                      name=f"psc_{hp}_{qc}_{j}")
                    for ab in range(2):
                        nc.tensor.matmul(
                            psc[:, ab, off:off + N],
                            KT_h[hp][64 * ab:64 * (ab + 1), j * P:(j + 1) * P],
                            QT_h[hp][64 * ab:64 * (ab + 1), qs:qs + N],
                            start=True, stop=True,
                            tile_position=(64 * ab, 0))
                    probs = pr.tile([P, 2, 512], bf16, tag="probs",
                                    name=f"pb_{hp}_{qc}_{j}")
                    nc.scalar.activation(probs[:, :, off:off + N],
                                         psc[:, :, off:off + N],
                                         AF.Exp, scale=0.125)
                    if qs == qsj:
                        nc.vector.tensor_tensor(
                            probs[:, :, off:off + P],
                            probs[:, :, off:off + P],
                            masks[:, j % 2, None, :].to_broadcast(
                                (P, 2, P)), ALU.mult)
                    for ab in range(2):
                        nc.tensor.matmul(
                            po_t[ab][0:65, off:off + N],
                            V[:, j, 2 * hp + ab, :],
                            probs[:, ab, off:off + N],
                            start=(j == 0), stop=(j == jmax - 1))
                # evict unnormalized out^T and stash the row-sums (row 64)
                for ab in range(2):
                    nc.vector.tensor_copy(
                        out=outT_g[qc][64 * ab:64 * (ab + 1), hp, :],
                        in_=po_t[ab][0:64, :])
                    rstmp = rsn.tile([1, 512], bf16, tag="rstmp",
                                     name=f"rst_{hp}_{qc}_{ab}")
                    nc.vector.tensor_copy(out=rstmp[:],
                                          in_=po_t[ab][64:65, :])
                    k = hp * 4 + qc * 2 + ab
                    nc.sync.dma_start(rs16_pad[k:k + 1, :], rstmp[:])

        for _pool in (pso_ps, psc_ps):
            _pool.release()

        # ---- deferred normalization of outT ------------------------------
        nrm_ps = tc.alloc_tile_pool(name="nrm", bufs=2, space="PSUM")
        rsT = rsn.tile([P, 4, P], bf16, name="rsT")
        for c in range(4):
            nc.scalar.dma_start_transpose(rsT[:, c, :],
                                          rs16_pad[:, c * P:(c + 1) * P])
        rsTf = rsn.tile([P, 4 * P], f32, name="rsTf")
        nc.vector.tensor_copy(out=rsTf[:], in_=rsT[:])
        recTf = rsn.tile([P, 4 * P], f32, name="recTf")
        nc.vector.reciprocal(recTf[:], rsTf[:])
        recT16 = rsn.tile([P, 4, P], bf16, name="recT16")
        nc.vector.tensor_copy(out=recT16[:], in_=recTf[:])
        rec16_pad = rsn.tile([P, 512], bf16, name="rec16_pad")
        for c in range(4):
            nc.scalar.dma_start_transpose(rec16_pad[:, c * P:(c + 1) * P],
                                          recT16[:, c, :])
        for qc in range(2):
            for hp in range(HP):
                pb = nrm_ps.tile([P, 512], f32, tag="pbn",
                                 name=f"pbn_{hp}_{qc}")
                nc.tensor.matmul(pb[:], eabt[:, hp * 2 + qc, :],
                                 rec16_pad[0:24, :],
                                 start=True, stop=True)
                nc.vector.tensor_tensor(
                    outT_g[qc][:, hp, :], outT_g[qc][:, hp, :],
                    pb[:], ALU.mult)

        for _pool in (nrm_ps, rsn, pr):
            _pool.release()
        pq_ps.release()
        w13.release()
        attnp.release()

        # -------- Phase 4: projection + residual + LN2 --------------------
        # FFN weights prefetch during proj (pool allocated below the proj
        # transients so it survives into the FFN phase)
        ffn = tc.alloc_tile_pool(name="ffn", bufs=1)
        w1_t = ffn.tile([P, EO, FF], bf16)
        for eo in range(EO):
            nc.gpsimd.dma_start(
                w1_t[:, eo, :], w1_d.ap()[eo * P:(eo + 1) * P, :])
        w2_t = ffn.tile([P, MB, C], bf16)
        for mp in range(MB):
            nc.gpsimd.dma_start(
                w2_t[:, mp, :], w2_d.ap()[mp * P:(mp + 1) * P, :])
        uT_g = [ffn.tile([P, MB, 512], bf16, tag=f"uT{g}", name=f"uT{g}")
                for g in range(2)]

        lnp2 = tc.alloc_tile_pool(name="lnp2", bufs=3)
        lns2 = tc.alloc_tile_pool(name="lns2", bufs=4)
        xres = tc.alloc_tile_pool(name="xres", bufs=3)
        ppr_ps = tc.alloc_tile_pool(name="ppr", bufs=2, space="PSUM")
        warm2_ps = tc.alloc_tile_pool(name="warm2_ps", bufs=1, space="PSUM")

        warm2 = warm2_ps.tile([P, 512], f32, tag="warm2", name="warm2")
        for qb in range(NQ):
            xqb = xres.tile([P, C], f32, tag="xqb", name=f"xqb_{qb}")
            nc.sync.dma_start(xqb[:], xq_d.ap()[qb * P:(qb + 1) * P, :])
            x2b = xres.tile([P, C], f32, tag="x2b", name=f"x2b_{qb}")
            for fo in range(2):
                pp = ppr_ps.tile([P, 384], f32, tag="ppr", name=f"pp_{qb}_{fo}")
                for fp in range(EO):
                    nc.tensor.matmul(
                        pp[:],
                        outT_g[qb // 4][:, fp, (qb % 4) * P:(qb % 4 + 1) * P],
                        wp_t[:, fp, fo * 384:(fo + 1) * 384],
                        start=(fp == 0), stop=False)
                nc.tensor.matmul(pp[:], ones1[:],
                                 bpr[:, fo * 384:(fo + 1) * 384],
                                 start=False, stop=True)
                nc.vector.tensor_tensor(
                    x2b[:, fo * 384:(fo + 1) * 384], pp[:],
                    xqb[:, fo * 384:(fo + 1) * 384], ALU.add)
            nc.sync.dma_start(x2_d.ap()[:, qb, :], x2b[:])
            ln_block(x2b[:], h2T_g[qb // 4], (qb % 4) * P,
                     f"x2{qb}", lnp2, lns2, sq_on_dve=True,
                     warm=warm2)

        for _pool in (warm2_ps, ppr_ps, xres, lns2, lnp2):
            _pool.release()

        # -------- Phase 5: FFN --------------------------------------------
        oup = tc.alloc_tile_pool(name="oup", bufs=3)
        pu_ps = tc.alloc_tile_pool(name="pu", bufs=2, space="PSUM")
        py_ps = tc.alloc_tile_pool(name="py", bufs=2, space="PSUM")

        for qc2 in range(2):
            for mb in range(MB):
                pu = pu_ps.tile([P, 512], f32, tag="pu", name=f"pu_{mb}_{qc2}")
                for eo in range(EO):
                    nc.tensor.matmul(
                        pu[:], w1_t[:, eo, mb * P:(mb + 1) * P],
                        h2T_g[qc2][:, eo, :],
                        start=(eo == 0), stop=(eo == EO - 1))
                nc.vector.tensor_scalar(
                    uT_g[qc2][:, mb, :], pu[:],
                    b1c[:, mb:mb + 1], 0.0, ALU.add, ALU.max)
        for qb in range(NQ):
            x2r = oup.tile([P, C], f32, tag="x2r", name=f"x2r_{qb}")
            nc.sync.dma_start(x2r[:], x2_d.ap()[:, qb, :])
            for fo in range(2):
                py = py_ps.tile([P, 384], f32, tag="py", name=f"py_{qb}_{fo}")
                for mp in range(MB):
                    nc.tensor.matmul(
                        py[:],
                        uT_g[qb // 4][:, mp, (qb % 4) * P:(qb % 4 + 1) * P],
                        w2_t[:, mp, fo * 384:(fo + 1) * 384],
                        start=(mp == 0), stop=False)
                nc.tensor.matmul(py[:], ones1[:],
                                 b2r[:, fo * 384:(fo + 1) * 384],
                                 start=False, stop=True)
                ot = oup.tile([P, 384], f32, tag="ot", name=f"ot_{qb}_{fo}")
                nc.vector.tensor_tensor(
                    ot[:], py[:], x2r[:, fo * 384:(fo + 1) * 384], ALU.add)
                nc.sync.dma_start(
                    out_d.ap()[qb * P:(qb + 1) * P,
                               fo * 384:(fo + 1) * 384], ot[:])

        for _pool in (py_ps, pu_ps, oup, ffn, late, pers):
            _pool.release()

    return nc


def prepare_in_maps(inputs):
    """Build the 8 per-core input maps from the full problem inputs."""
    x = np.asarray(inputs["x"], np.float32)
    wq = np.asarray(inputs["wq"], np.float32)
    wk = np.asarray(inputs["wk"], np.float32)
    wv = np.asarray(inputs["wv"], np.float32)
    w_proj = np.asarray(inputs["w_proj"], np.float32)
    b_proj = np.asarray(inputs["b_proj"], np.float32)
    w1 = np.asarray(inputs["w1"], np.float32)
    b1 = np.asarray(inputs["b1"], np.float32)
    w2 = np.asarray(inputs["w2"], np.float32)
    b2 = np.asarray(inputs["b2"], np.float32)
    g1 = np.asarray(inputs["ln1_g"], np.float32)
    be1 = np.asarray(inputs["ln1_b"], np.float32)
    g2 = np.asarray(inputs["ln2_g"], np.float32)
    be2 = np.asarray(inputs["ln2_b"], np.float32)

    bf = ml_dtypes.bfloat16
    wq_r = wq.transpose(1, 0, 2).reshape(C, C)       # [c, h*d]
    wk_r = wk.transpose(1, 0, 2).reshape(C, C)
    wv_r = wv.transpose(1, 0, 2).reshape(C, C)
    wq_g = (g1[:, None] * wq_r).astype(bf)
    wk_g = (g1[:, None] * wk_r).astype(bf)
    wv_g = (g1[:, None] * wv_r).astype(bf)
    bq = (be1 @ wq_r).reshape(HP, P).T.copy().astype(np.float32)   # [128, hp]
    bk = (be1 @ wk_r).reshape(HP, P).T.copy().astype(np.float32)
    bv = (be1 @ wv_r).reshape(1, C).astype(bf)
    w1_g = (g2[:, None] * w1).astype(bf)
    b1f = (b1 + be2 @ w1).reshape(MB, P).T.copy().astype(np.float32)  # [128, mb]
    wp16 = w_proj.astype(bf)
    w2_16 = w2.astype(bf)
    bp = b_proj.reshape(1, C).astype(bf)
    b2r = b2.reshape(1, C).astype(bf)

    ci = np.arange(P)[:, None]
    qi = np.arange(P)[None, :]
    tri = (ci <= qi).astype(np.float32)          # visible where c <= q
    m_par = [
        np.stack([tri, np.zeros((P, P), np.float32)], 0),   # parity 0
        np.stack([np.ones((P, P), np.float32), tri], 0),    # parity 1
    ]

    eab = np.zeros((12, 24, P), np.float32)
    for i in range(12):
        hp_, qc_ = i // 2, i % 2
        eab[i, hp_ * 4 + qc_ * 2 + 0, 0:64] = 1.0
        eab[i, hp_ * 4 + qc_ * 2 + 1, 64:128] = 1.0
    eab16 = eab.astype(bf)

    in_maps = []
    for core in range(8):
        b, p = core // 2, core % 2
        xf = np.ascontiguousarray(x[b])
        xq = np.ascontiguousarray(
            x[b].reshape(NB, P, C)[p::2].reshape(TQ, C))
        mk = np.ascontiguousarray(m_par[p].transpose(1, 0, 2)).astype(bf)
        s0 = np.full((P, 1), 1.0 - p, np.float32)
        s1 = np.full((P, 1), float(p), np.float32)
        in_maps.append({
            "xq": xq, "xf": xf,
            "wq": wq_g, "wk": wk_g, "wv": wv_g, "wp": wp16,
            "w1": w1_g, "w2": w2_16,
            "bq": bq, "bk": bk, "bv": bv, "bp": bp, "b1": b1f, "b2": b2r,
            "mk": mk, "s0": s0, "s1": s1, "eab": eab16,
        })
    return in_maps


def assemble_output(results):
    """Reassemble the 8 per-core [1024, 768] outputs into [4, 2048, 768]."""
    out = np.empty((4, T, C), np.float32)
    for core in range(8):
        b, p = core // 2, core % 2
        blocks = results[core]["out"].reshape(NQ, P, C)
        ov = out[b].reshape(NB, P, C)
        ov[p::2] = blocks
    return out


_CACHED_NC = None


def kernel(**inputs) -> np.ndarray:
    global _CACHED_NC
    from concourse.bass_utils import run_bass_kernel_spmd

    if _CACHED_NC is None:
        nc = build_program()
        split_multiwait_instructions(nc)
        _CACHED_NC = nc
    in_maps = prepare_in_maps(inputs)
    res = run_bass_kernel_spmd(_CACHED_NC, in_maps, list(range(8)))
    return assemble_output(res.results)
